# revision 1
# baseline (speedup 1.0000x reference)
"""GAT (2-layer, PyG-style) Bass kernel for Trainium2, 8 NeuronCores.

Sharding: 1D destination-node partition. Each core owns N/8 dst nodes; edges
are bucketed by dst so segment-softmax and scatter-add are local. Layer-1
node features (h | a_src | a_dst) are computed redundantly per core into a
per-core-permuted table (own shard first, so dst-side rows fit int16); the
layer-2 table is shard-computed and AllGathered.

Per-edge row fetches use dma_gather (int16 indices, 256B-multiple rows);
src-side indices >= 32768 are handled by a per-group section split over two
table views. Segmented softmax + scatter-add go through a one-hot indicator
matmul on the tensor engine.
"""

import os
import sys

sys.path.insert(0, "/opt/trn_rl_repo")

import numpy as np
import ml_dtypes

BF16 = ml_dtypes.bfloat16

from concourse import bacc, bass, mybir, tile
from concourse.bass_utils import run_bass_kernel_spmd

AF = mybir.ActivationFunctionType
ALU = mybir.AluOpType
DT_BF16 = mybir.dt.bfloat16
DT_F32 = mybir.dt.float32
DT_I16 = mybir.dt.int16
HALF = 32768


class Cfg:
    def __init__(self, N=50000, E=800000):
        self.N = N
        self.E = E
        self.NC = 8
        self.NFEAT = 256
        self.NHID = 16
        self.HEADS = 8
        self.NCLASS = 40
        self.FH = self.HEADS * self.NHID          # 128
        self.T1W = 256                            # table1 row: h|a_src|a_dst|pad
        self.T2W = 128                            # table2 row: h2|a_src2|a_dst2|pad
        assert N % self.NC == 0
        self.SHARD = N // self.NC
        self.NDT = (self.SHARD + 127) // 128      # dst tiles per core
        self.LSH = self.NDT * 128
        self.NROW2 = self.NC * self.LSH
        self.ABATCH = 3
        self.NA = ((N + 384 - 1) // 384) * 384    # stage-A padded rows
        self.GG = 2                               # dst tiles per gather group



CHUNK = 1024


def _gather_chunked(nc, out_tile, col0, table_view, idx_tile, o16, total, elem):
    """Emit dma_gather calls of <=CHUNK idxs; out columns start at col0."""
    done = 0
    while done < total:
        n = min(CHUNK, total - done)
        nc.gpsimd.dma_gather(
            out_tile[:, (col0 + done) // 128:(col0 + done + n) // 128, :],
            table_view,
            idx_tile[:, o16 + done // 16:o16 + (done + n) // 16],
            n, n, elem)
        done += n

def _wrap16(vals):
    """int16 values -> dma_gather wrapped layout [128, len/16]."""
    n = len(vals)
    assert n % 16 == 0
    w = np.asarray(vals, np.int16).reshape(n // 16, 16).T  # [16, n/16]
    return np.tile(w, (8, 1))                              # [128, n/16]


def _prep_edges(cfg, edge_index):
    """Per-core, per-group edge layout with 4 sections by
    (src row>=32768 in table1, src row>=32768 in table2)."""
    N, NC, NDT, SHARD, LSH, GG = (cfg.N, cfg.NC, cfg.NDT, cfg.SHARD,
                                  cfg.LSH, cfg.GG)
    src = np.concatenate([np.asarray(edge_index[0]), np.arange(N)]).astype(np.int64)
    dst = np.concatenate([np.asarray(edge_index[1]), np.arange(N)]).astype(np.int64)
    core = dst // SHARD
    ldst = dst - core * SHARD                      # local dst 0..SHARD-1
    row2 = src + (src // SHARD) * (LSH - SHARD)    # table2 row of src
    groups = [(g, min(g + GG, NDT)) for g in range(0, NDT, GG)]
    NG = len(groups)
    tl = ldst // 128
    gl = np.searchsorted(np.array([a for a, _ in groups]), tl, side="right") - 1

    # per-core src row in the permuted table1: own shard first, others by id
    row1 = np.empty((NC, N), np.int64)
    for k in range(NC):
        own = np.arange(k * SHARD, (k + 1) * SHARD)
        other = np.concatenate([np.arange(0, k * SHARD),
                                np.arange((k + 1) * SHARD, N)])
        perm = np.concatenate([own, other])
        inv = np.empty(N, np.int64)
        inv[perm] = np.arange(N)
        row1[k] = inv

    key = core * NG + gl
    order = np.argsort(key, kind="stable")
    ks = key[order]
    bounds = np.searchsorted(ks, np.arange(NC * NG + 1))

    # section membership per (core, edge): 2*(row1>=H) + (row2>=H)
    # first pass: section counts -> shared section sizes CS[g, s]
    secs = [[None] * NG for _ in range(NC)]
    cnt = np.zeros((NC, NG, 4), np.int64)
    for k in range(NC):
        for g in range(NG):
            ids = order[bounds[k * NG + g]:bounds[k * NG + g + 1]]
            s = 2 * (row1[k][src[ids]] >= HALF) + (row2[ids] >= HALF)
            # order: section, then tile, stable
            o2 = np.lexsort((tl[ids], s))
            ids = ids[o2]
            s = s[o2]
            secs[k][g] = (ids, s)
            cnt[k, g] = np.bincount(s, minlength=4)
    CS = (-(-cnt.max(axis=0) // 128) * 128).astype(np.int64)  # [NG, 4]
    Call = CS.sum(axis=1)                                     # slots per group
    NCOL = (Call // 128).astype(np.int64)

    # blob layout per group (int16 cols): src1lo, src1hi, dst, s2_0..3
    # col counts: (CS0+CS1)/16, (CS2+CS3)/16, Call/16, CS0/16.. CS3/16
    blob_cols = ((CS[:, 0] + CS[:, 1]) + (CS[:, 2] + CS[:, 3])
                 + Call + Call) // 16
    blob_off = np.concatenate([[0], np.cumsum(blob_cols)]).astype(int)
    BLOBTOT = int(blob_off[-1])
    lloc_off = np.concatenate([[0], np.cumsum(NCOL)]).astype(int)
    LLTOT = int(lloc_off[-1])

    blob = np.zeros((NC, 128, BLOBTOT), np.int16)
    llocb = np.full((NC, 128, LLTOT), 1000.0, np.float32)

    # per (group, tile): column ranges [(c0, c1), ...] in group-local columns
    tile_ranges = [[[] for _ in range(NDT)] for _ in range(NG)]

    for k in range(NC):
        for g, (glo, ghi) in enumerate(groups):
            ids, s = secs[k][g]
            # slot position: section base + within-section position
            sbase = np.concatenate([[0], np.cumsum(CS[g])])[:4]
            pos = np.empty(len(ids), np.int64)
            for sec in range(4):
                m = s == sec
                pos[m] = sbase[sec] + np.arange(m.sum())
            C = int(Call[g])
            # index arrays, padded with 0 (valid row 0)
            r1 = np.zeros(C, np.int64)
            r2v = np.zeros(C, np.int64)
            dl = np.zeros(C, np.int64)
            lv = np.full(C, 1000.0, np.float32)
            r1[pos] = row1[k][src[ids]]
            r2v[pos] = row2[ids]
            dl[pos] = ldst[ids]
            lv[pos] = (ldst[ids] - glo * 128).astype(np.float32)
            c01 = int(CS[g, 0] + CS[g, 1])
            seg = []
            seg.append(_wrap16(r1[:c01]))                        # src1 low
            seg.append(_wrap16(r1[c01:] - HALF * (r1[c01:] >= HALF)))  # src1 hi
            seg.append(_wrap16(dl))                              # dst (both layers)
            cb = np.concatenate([[0], np.cumsum(CS[g])]).astype(int)
            for sec in range(4):
                v = r2v[cb[sec]:cb[sec + 1]]
                seg.append(_wrap16(v - HALF * (v >= HALF)))
            blob[k, :, blob_off[g]:blob_off[g + 1]] = np.concatenate(seg, axis=1)
            llocb[k, :, lloc_off[g]:lloc_off[g + 1]] = \
                lv.reshape(int(NCOL[g]), 128).T
            if k == 0:
                # column ranges per tile (shared: derive from all cores below)
                pass
    # tile column ranges: union over cores of occupied columns per (g, t)
    occ = np.zeros((NG, NDT, 1), object)
    for g, (glo, ghi) in enumerate(groups):
        ncol = int(NCOL[g])
        used = np.zeros((NDT, ncol), bool)
        for k in range(NC):
            ids, s = secs[k][g]
            sbase = np.concatenate([[0], np.cumsum(CS[g])])[:4]
            pos = np.empty(len(ids), np.int64)
            for sec in range(4):
                m = s == sec
                pos[m] = sbase[sec] + np.arange(m.sum())
            t_of = tl[ids]
            for t in range(glo, ghi):
                cols = np.unique(pos[t_of == t] // 128)
                used[t, cols] = True
        for t in range(glo, ghi):
            cols = np.where(used[t])[0]
            ranges = []
            if len(cols):
                brk = np.where(np.diff(cols) > 1)[0]
                st = 0
                for b in list(brk) + [len(cols) - 1]:
                    ranges.append((int(cols[st]), int(cols[b]) + 1))
                    st = b + 1
            tile_ranges[g][t] = ranges

    meta = dict(groups=groups, CS=CS, Call=Call, NCOL=NCOL,
                blob_off=blob_off, BLOBTOT=BLOBTOT,
                lloc_off=lloc_off, LLTOT=LLTOT, tile_ranges=tile_ranges)
    return meta, blob, llocb.astype(BF16)


def _prep_weights(cfg, W1, att_src1, att_dst1, b1, W2, att_src2, att_dst2, b2):
    W1 = np.asarray(W1, np.float32)
    A1 = np.zeros((cfg.FH, 2 * cfg.HEADS), np.float32)
    for h in range(cfg.HEADS):
        A1[h * cfg.NHID:(h + 1) * cfg.NHID, h] = np.asarray(att_src1)[h]
        A1[h * cfg.NHID:(h + 1) * cfg.NHID, cfg.HEADS + h] = np.asarray(att_dst1)[h]
    W1cat = np.concatenate([W1, W1 @ A1], axis=1).astype(BF16)  # [NFEAT, 144]
    W2cat = np.zeros((cfg.FH, 48), np.float32)
    W2cat[:, :cfg.NCLASS] = np.asarray(W2)
    W2cat[:, cfg.NCLASS] = np.asarray(W2) @ np.asarray(att_src2)[0]
    W2cat[:, cfg.NCLASS + 1] = np.asarray(W2) @ np.asarray(att_dst2)[0]
    W2cat = W2cat.astype(BF16)
    b1rep = np.broadcast_to(np.asarray(b1, np.float32), (128, cfg.FH)).copy()
    b2rep = np.broadcast_to(np.asarray(b2, np.float32), (128, cfg.NCLASS)).copy()
    return W1cat, W2cat, b1rep, b2rep


def build_bass(cfg, meta):
    nc = bacc.Bacc("TRN2", target_bir_lowering=False, debug=False)
    NDT, GG = cfg.NDT, cfg.GG
    TBW = cfg.FH + 2 * cfg.HEADS                   # 144 live cols of table1
    NAB = cfg.NA // (128 * cfg.ABATCH)
    groups = meta["groups"]
    CS, Call, NCOL = meta["CS"], meta["Call"], meta["NCOL"]
    blob_off, lloc_off = meta["blob_off"], meta["lloc_off"]
    tile_ranges = meta["tile_ranges"]
    NCOLMAX = int(max(NCOL))
    BLOBMAX = int(max(blob_off[i + 1] - blob_off[i] for i in range(len(groups))))
    CMAX_T = 1
    for g in range(len(groups)):
        for t in range(NDT):
            if tile_ranges[g][t]:
                CMAX_T = max(CMAX_T,
                             sum(b - a for (a, b) in tile_ranges[g][t]))

    xT = nc.dram_tensor("xT", [cfg.NFEAT, cfg.NA], DT_BF16, kind="ExternalInput")
    w1cat = nc.dram_tensor("w1cat", [cfg.NFEAT, TBW], DT_BF16, kind="ExternalInput")
    w2cat = nc.dram_tensor("w2cat", [cfg.FH, 48], DT_BF16, kind="ExternalInput")
    b1rep_d = nc.dram_tensor("b1rep", [128, cfg.FH], DT_F32, kind="ExternalInput")
    b2rep_d = nc.dram_tensor("b2rep", [128, cfg.NCLASS], DT_F32, kind="ExternalInput")
    iota_d = nc.dram_tensor("iota", [128, GG * 128], DT_BF16, kind="ExternalInput")
    ident_d = nc.dram_tensor("ident", [128, 128], DT_F32, kind="ExternalInput")
    blob_d = nc.dram_tensor("blob", [128, meta["BLOBTOT"]], DT_I16,
                            kind="ExternalInput")
    lloc_d = nc.dram_tensor("lloc", [128, meta["LLTOT"]], DT_BF16,
                            kind="ExternalInput")
    out_d = nc.dram_tensor("out", [cfg.LSH, cfg.NCLASS], DT_F32,
                           kind="ExternalOutput")

    table1 = nc.dram_tensor("table1", [cfg.NA, cfg.T1W], DT_BF16)
    adst1 = nc.dram_tensor("adst1", [cfg.LSH, 128], DT_BF16)
    h2own = nc.dram_tensor("h2own", [cfg.LSH, cfg.T2W], DT_BF16)
    h2own_c = nc.dram_tensor("h2own_c", [cfg.LSH, 48], DT_BF16)
    table2s = nc.dram_tensor("table2s", [cfg.NROW2, 48], DT_BF16,
                             addr_space="Shared")
    table2 = nc.dram_tensor("table2", [cfg.NROW2, cfg.T2W], DT_BF16)

    dbg = os.environ.get("GAT_DEBUG_DUMP") == "1"
    if dbg:
        dbg_t1 = nc.dram_tensor("dbg_t1", [cfg.NA, cfg.T1W], DT_BF16,
                                kind="ExternalOutput")
        dbg_t2 = nc.dram_tensor("dbg_t2", [cfg.NROW2, cfg.T2W], DT_BF16,
                                kind="ExternalOutput")
        dbg_ps = nc.dram_tensor("dbg_ps", [cfg.LSH, TBW], DT_F32,
                                kind="ExternalOutput")
        dbg_g1 = nc.dram_tensor("dbg_g1", [128, NCOLMAX, cfg.T1W], DT_BF16,
                                kind="ExternalOutput")
        dbg_gd = nc.dram_tensor("dbg_gd", [128, NCOLMAX, 128], DT_BF16,
                                kind="ExternalOutput")

    with tile.TileContext(nc) as tc:
        with tc.tile_pool(name="const", bufs=1) as cpool:
            w1_sb = cpool.tile([128, cfg.NFEAT // 128, TBW], DT_BF16)
            nc.sync.dma_start(out=w1_sb[:],
                              in_=w1cat[:].rearrange("(kt p) c -> p kt c", p=128))
            w2_sb = cpool.tile([128, 48], DT_BF16)
            nc.sync.dma_start(out=w2_sb[:], in_=w2cat[:])
            b1_sb = cpool.tile([128, cfg.FH], DT_F32)
            nc.sync.dma_start(out=b1_sb[:], in_=b1rep_d[:])
            b2_sb = cpool.tile([128, cfg.NCLASS], DT_F32)
            nc.sync.dma_start(out=b2_sb[:], in_=b2rep_d[:])
            iota_sb = cpool.tile([128, GG * 128], DT_BF16)
            nc.sync.dma_start(out=iota_sb[:], in_=iota_d[:])
            ident_sb = cpool.tile([128, 128], DT_F32)
            nc.sync.dma_start(out=ident_sb[:], in_=ident_d[:])
            outf_sb = cpool.tile([128, NDT, cfg.NCLASS], DT_F32)

            # ============ stage A: table1 + local a_dst table ================
            with (
                tc.tile_pool(name="ax", bufs=3) as axp,
                tc.tile_pool(name="atb", bufs=3) as atbp,
                tc.tile_pool(name="apsum", bufs=2, space="PSUM") as app,
            ):
                for bidx in range(NAB):
                    n0 = bidx * 128 * cfg.ABATCH
                    xt = axp.tile([128, cfg.NFEAT // 128, 128 * cfg.ABATCH],
                                  DT_BF16, tag="xt")
                    for kt in range(cfg.NFEAT // 128):
                        nc.sync.dma_start(
                            out=xt[:, kt, :],
                            in_=xT[kt * 128:(kt + 1) * 128,
                                   n0:n0 + 128 * cfg.ABATCH])
                    pa = app.tile([128, cfg.ABATCH * TBW], DT_F32, tag="pa")
                    for m in range(cfg.ABATCH):
                        for kt in range(cfg.NFEAT // 128):
                            nc.tensor.matmul(
                                out=pa[:, m * TBW:(m + 1) * TBW],
                                lhsT=xt[:, kt, m * 128:(m + 1) * 128],
                                rhs=w1_sb[:, kt, :],
                                start=(kt == 0),
                                stop=(kt == cfg.NFEAT // 128 - 1))
                    tb = atbp.tile([128, cfg.ABATCH * TBW], DT_BF16, tag="tb")
                    nc.vector.tensor_copy(out=tb[:], in_=pa[:])
                    nc.sync.dma_start(
                        out=table1[n0:n0 + 128 * cfg.ABATCH, 0:TBW].rearrange(
                            "(m p) c -> p m c", p=128),
                        in_=tb[:].rearrange("p (m c) -> p m c", c=TBW))
                    # local a_dst rows (a_dst = psum cols FH+8 : FH+16)
                    for m in range(cfg.ABATCH):
                        r0 = n0 + m * 128
                        if r0 >= cfg.LSH:
                            break
                        ad = atbp.tile([128, cfg.HEADS], DT_BF16, tag="ad")
                        nc.vector.tensor_copy(
                            out=ad[:],
                            in_=pa[:, m * TBW + cfg.FH + cfg.HEADS:
                                   m * TBW + cfg.FH + 2 * cfg.HEADS])
                        nc.sync.dma_start(out=adst1[r0:r0 + 128, 0:cfg.HEADS],
                                          in_=ad[:])

            tc.strict_bb_all_engine_barrier()

            # ============ stage B/C: layer-1 edges + layer-2 table ===========
            with (
                tc.tile_pool(name="gx", bufs=2) as gxp,
                tc.tile_pool(name="gi", bufs=2) as gip,
                tc.tile_pool(name="mm", bufs=2) as mmp,
                tc.tile_pool(name="rh", bufs=2) as rhp,
                tc.tile_pool(name="sm", bufs=3) as smp,
                tc.tile_pool(name="cc", bufs=2) as ccp,
                tc.tile_pool(name="ps1", bufs=2, space="PSUM") as ps1,
                tc.tile_pool(name="ps2", bufs=2, space="PSUM") as ps2,
                tc.tile_pool(name="ps3", bufs=2, space="PSUM") as ps3,
            ):
                def load_idx(g):
                    bo = int(blob_off[g])
                    bw = int(blob_off[g + 1]) - bo
                    idx = gip.tile([128, BLOBMAX], DT_I16, tag="idx")
                    nc.sync.dma_start(out=idx[:, :bw], in_=blob_d[:, bo:bo + bw])
                    ll = gip.tile([128, NCOLMAX], DT_BF16, tag="ll")
                    lo = int(lloc_off[g])
                    lw = int(lloc_off[g + 1]) - lo
                    nc.sync.dma_start(out=ll[:, :lw], in_=lloc_d[:, lo:lo + lw])
                    return idx, ll

                def build_M(g, t, glo, ll):
                    ranges = tile_ranges[g][t]
                    ncols_t = sum(b - a for (a, b) in ranges)
                    M = mmp.tile([128, max(CMAX_T, 1), 128], DT_BF16, tag="M")
                    cpos = 0
                    cols = []
                    for (a, b) in ranges:
                        w = b - a
                        nc.vector.tensor_tensor(
                            out=M[:, cpos:cpos + w, :],
                            in0=ll[:, a:b].to_broadcast([128, w, 128]),
                            in1=iota_sb[:, (t - glo) * 128:(t - glo + 1) * 128]
                                .unsqueeze(1).to_broadcast([128, w, 128]),
                            op=ALU.is_equal)
                        for c in range(a, b):
                            cols.append((cpos + c - a, c))
                        cpos += w
                    return M, cols, ncols_t

                for g, (glo, ghi) in enumerate(groups):
                    C = int(Call[g])
                    ncol = int(NCOL[g])
                    cs = [int(v) for v in CS[g]]
                    c01 = cs[0] + cs[1]
                    c23 = cs[2] + cs[3]
                    idx, ll = load_idx(g)
                    # offsets into idx blob (cols of 16 idx each)
                    o = 0
                    o_s1lo = o; o += c01 // 16
                    o_s1hi = o; o += c23 // 16
                    o_dst = o; o += C // 16
                    o_s2 = []
                    for sec in range(4):
                        o_s2.append(o); o += cs[sec] // 16
                    g1 = gxp.tile([128, NCOLMAX, cfg.T1W], DT_BF16, tag="g1")
                    if c01:
                        _gather_chunked(nc, g1, 0,
                                        table1[0:min(HALF, cfg.NA), :],
                                        idx, o_s1lo, c01, cfg.T1W)
                    if c23:
                        _gather_chunked(nc, g1, c01, table1[HALF:cfg.NA, :],
                                        idx, o_s1hi, c23, cfg.T1W)
                    gd = gxp.tile([128, NCOLMAX, 128], DT_BF16, tag="gd")
                    _gather_chunked(nc, gd, 0, adst1[:], idx, o_dst, C, 128)
                    if dbg and g == 0:
                        nc.sync.dma_start(out=dbg_g1[:, :ncol, :],
                                          in_=g1[:, :ncol, :])
                        nc.sync.dma_start(out=dbg_gd[:, :ncol, :],
                                          in_=gd[:, :ncol, :])
                    # edgewise: logits -> leaky -> exp ; messages
                    lg = smp.tile([128, NCOLMAX, cfg.HEADS], DT_BF16, tag="lg")
                    nc.vector.tensor_tensor(
                        out=lg[:, :ncol, :],
                        in0=g1[:, :ncol, cfg.FH:cfg.FH + cfg.HEADS],
                        in1=gd[:, :ncol, 0:cfg.HEADS], op=ALU.add)
                    lr = smp.tile([128, NCOLMAX, cfg.HEADS], DT_BF16, tag="lr")
                    nc.vector.tensor_scalar_mul(
                        out=lr[:, :ncol, :], in0=lg[:, :ncol, :], scalar1=0.2)
                    nc.vector.tensor_tensor(
                        out=lr[:, :ncol, :], in0=lr[:, :ncol, :],
                        in1=lg[:, :ncol, :], op=ALU.max)
                    rhs = rhp.tile([128, NCOLMAX, cfg.FH + cfg.HEADS], DT_BF16,
                                   tag="rhs")
                    nc.scalar.activation(
                        out=rhs[:, :ncol, cfg.FH:], in_=lr[:, :ncol, :],
                        func=AF.Exp)
                    for hh in range(cfg.HEADS):
                        nc.vector.tensor_tensor(
                            out=rhs[:, :ncol, hh * cfg.NHID:(hh + 1) * cfg.NHID],
                            in0=g1[:, :ncol, hh * cfg.NHID:(hh + 1) * cfg.NHID],
                            in1=rhs[:, :ncol, cfg.FH + hh:cfg.FH + hh + 1]
                                .to_broadcast([128, ncol, cfg.NHID]),
                            op=ALU.mult)
                    for t in range(glo, ghi):
                        M, cols, nct = build_M(g, t, glo, ll)
                        pseg = ps1.tile([128, TBW], DT_F32, tag="pseg")
                        for i, (mc, c) in enumerate(cols):
                            nc.tensor.matmul(
                                out=pseg[:, 0:cfg.FH + cfg.HEADS],
                                lhsT=M[:, mc, :], rhs=rhs[:, c, :],
                                start=(i == 0), stop=(i == len(cols) - 1))
                        if dbg:
                            psc = ccp.tile([128, TBW], DT_F32, tag="psc")
                            nc.vector.tensor_copy(out=psc[:], in_=pseg[:])
                            nc.sync.dma_start(
                                out=dbg_ps[t * 128:(t + 1) * 128, :], in_=psc[:])
                        # ---- stage C ----
                        rec = ccp.tile([128, cfg.HEADS], DT_F32, tag="rec")
                        nc.vector.reciprocal(
                            out=rec[:], in_=pseg[:, cfg.FH:cfg.FH + cfg.HEADS])
                        o1 = ccp.tile([128, cfg.FH], DT_F32, tag="o1")
                        nc.vector.tensor_tensor(
                            out=o1[:].rearrange("p (h c) -> p h c", c=cfg.NHID),
                            in0=pseg[:, 0:cfg.FH].rearrange(
                                "p (h c) -> p h c", c=cfg.NHID),
                            in1=rec[:].to_broadcast([128, cfg.HEADS, cfg.NHID]),
                            op=ALU.mult)
                        nc.vector.tensor_tensor(
                            out=o1[:], in0=o1[:], in1=b1_sb[:], op=ALU.add)
                        tmin = ccp.tile([128, cfg.FH], DT_F32, tag="tmin")
                        nc.vector.tensor_scalar_min(
                            out=tmin[:], in0=o1[:], scalar1=0.0)
                        nc.scalar.activation(out=tmin[:], in_=tmin[:],
                                             func=AF.Exp)
                        nc.vector.tensor_scalar_add(
                            out=tmin[:], in0=tmin[:], scalar1=-1.0)
                        a1t = ccp.tile([128, cfg.FH], DT_F32, tag="a1t")
                        nc.vector.tensor_tensor(
                            out=a1t[:], in0=o1[:], in1=tmin[:], op=ALU.max)
                        ptr = ps2.tile([128, 128], DT_F32, tag="ptr")
                        nc.tensor.transpose(out=ptr[:], in_=a1t[:],
                                            identity=ident_sb[:])
                        a1T = ccp.tile([128, 128], DT_BF16, tag="a1T")
                        nc.scalar.copy(out=a1T[:], in_=ptr[:])
                        ph2 = ps3.tile([128, 48], DT_F32, tag="ph2")
                        nc.tensor.matmul(out=ph2[:], lhsT=a1T[:], rhs=w2_sb[:],
                                         start=True, stop=True)
                        t2 = ccp.tile([128, 48], DT_BF16, tag="t2")
                        nc.scalar.copy(out=t2[:], in_=ph2[:])
                        nc.sync.dma_start(
                            out=h2own_c[t * 128:(t + 1) * 128, :], in_=t2[:])
                        nc.sync.dma_start(
                            out=h2own[t * 128:(t + 1) * 128, 0:48], in_=t2[:])

                # ---- AllGather layer-2 table; widen rows to 128 ----
                nc.gpsimd.collective_compute(
                    "AllGather", ALU.bypass,
                    replica_groups=[list(range(cfg.NC))],
                    ins=[h2own_c[:]], outs=[table2s[:]])
                nc.sync.dma_start(out=table2[:, 0:48], in_=table2s[:])
                tc.strict_bb_all_engine_barrier()

                # ============ stage D/E: layer-2 edges =======================
                NCL = cfg.NCLASS
                for g, (glo, ghi) in enumerate(groups):
                    C = int(Call[g])
                    ncol = int(NCOL[g])
                    cs = [int(v) for v in CS[g]]
                    c01 = cs[0] + cs[1]
                    idx, ll = load_idx(g)
                    o = 0
                    o_s1lo = o; o += c01 // 16
                    o_s1hi = o; o += (cs[2] + cs[3]) // 16
                    o_dst = o; o += C // 16
                    o_s2 = []
                    for sec in range(4):
                        o_s2.append(o); o += cs[sec] // 16
                    g2 = gxp.tile([128, NCOLMAX, cfg.T2W], DT_BF16, tag="gd")
                    cb = 0
                    for sec in range(4):
                        if cs[sec] == 0:
                            continue
                        tv = table2[0:min(HALF, cfg.NROW2), :] if sec in (0, 2) \
                            else table2[HALF:cfg.NROW2, :]
                        _gather_chunked(nc, g2, cb, tv, idx, o_s2[sec],
                                        cs[sec], cfg.T2W)
                        cb += cs[sec]
                    gd2 = gxp.tile([128, NCOLMAX, cfg.T2W], DT_BF16, tag="gd2")
                    _gather_chunked(nc, gd2, 0, h2own[:], idx, o_dst, C,
                                    cfg.T2W)
                    lg = smp.tile([128, NCOLMAX, 1], DT_BF16, tag="lg")
                    nc.vector.tensor_tensor(
                        out=lg[:, :ncol, :],
                        in0=g2[:, :ncol, NCL:NCL + 1],
                        in1=gd2[:, :ncol, NCL + 1:NCL + 2], op=ALU.add)
                    lr = smp.tile([128, NCOLMAX, 1], DT_BF16, tag="lr")
                    nc.vector.tensor_scalar_mul(
                        out=lr[:, :ncol, :], in0=lg[:, :ncol, :], scalar1=0.2)
                    nc.vector.tensor_tensor(
                        out=lr[:, :ncol, :], in0=lr[:, :ncol, :],
                        in1=lg[:, :ncol, :], op=ALU.max)
                    rhs = rhp.tile([128, NCOLMAX, NCL + 1], DT_BF16, tag="rhs")
                    nc.scalar.activation(
                        out=rhs[:, :ncol, NCL:], in_=lr[:, :ncol, :],
                        func=AF.Exp)
                    nc.vector.tensor_tensor(
                        out=rhs[:, :ncol, 0:NCL],
                        in0=g2[:, :ncol, 0:NCL],
                        in1=rhs[:, :ncol, NCL:NCL + 1]
                            .to_broadcast([128, ncol, NCL]),
                        op=ALU.mult)
                    for t in range(glo, ghi):
                        M, cols, nct = build_M(g, t, glo, ll)
                        pseg = ps1.tile([128, NCL + 1], DT_F32, tag="pseg")
                        for i, (mc, c) in enumerate(cols):
                            nc.tensor.matmul(
                                out=pseg[:], lhsT=M[:, mc, :], rhs=rhs[:, c, :],
                                start=(i == 0), stop=(i == len(cols) - 1))
                        rec = ccp.tile([128, 1], DT_F32, tag="rec")
                        nc.vector.reciprocal(out=rec[:], in_=pseg[:, NCL:])
                        nc.vector.tensor_tensor(
                            out=outf_sb[:, t, :],
                            in0=pseg[:, 0:NCL],
                            in1=rec[:].to_broadcast([128, NCL]),
                            op=ALU.mult)
                        nc.vector.tensor_tensor(
                            out=outf_sb[:, t, :], in0=outf_sb[:, t, :],
                            in1=b2_sb[:], op=ALU.add)

            # ============ stage F: log_softmax ===========================
            with tc.tile_pool(name="fin", bufs=1) as fpp:
                mx = fpp.tile([128, NDT, 1], DT_F32, tag="mx")
                nc.vector.tensor_reduce(
                    out=mx[:], in_=outf_sb[:], axis=mybir.AxisListType.X,
                    op=ALU.max)
                ex = fpp.tile([128, NDT, cfg.NCLASS], DT_F32, tag="ex")
                nc.vector.tensor_tensor(
                    out=ex[:], in0=outf_sb[:],
                    in1=mx[:].to_broadcast([128, NDT, cfg.NCLASS]),
                    op=ALU.subtract)
                nc.scalar.activation(out=ex[:], in_=ex[:], func=AF.Exp)
                sm = fpp.tile([128, NDT, 1], DT_F32, tag="sm")
                nc.vector.tensor_reduce(
                    out=sm[:], in_=ex[:], axis=mybir.AxisListType.X,
                    op=ALU.add)
                nc.scalar.activation(out=sm[:], in_=sm[:], func=AF.Ln)
                nc.vector.tensor_tensor(
                    out=sm[:], in0=sm[:], in1=mx[:], op=ALU.add)
                nc.vector.tensor_tensor(
                    out=outf_sb[:], in0=outf_sb[:],
                    in1=sm[:].to_broadcast([128, NDT, cfg.NCLASS]),
                    op=ALU.subtract)
                nc.sync.dma_start(
                    out=out_d[:].rearrange("(t p) c -> p t c", p=128),
                    in_=outf_sb[:])
                if dbg:
                    nc.sync.dma_start(out=dbg_t1[:], in_=table1[:])
                    nc.sync.dma_start(out=dbg_t2[:], in_=table2[:])
    nc.compile()
    return nc


def _run(cfg, inputs, trace=False):
    meta, blob, llocb = _prep_edges(cfg, np.asarray(inputs["edge_index"]))
    W1cat, W2cat, b1rep, b2rep = _prep_weights(
        cfg, inputs["W1"], inputs["att_src1"], inputs["att_dst1"], inputs["b1"],
        inputs["W2"], inputs["att_src2"], inputs["att_dst2"], inputs["b2"])
    x = np.asarray(inputs["x"], np.float32)
    xTf = np.zeros((cfg.NFEAT, cfg.NA), BF16)
    xTf[:, :cfg.N] = x.T.astype(BF16)
    iota = np.broadcast_to(np.arange(cfg.GG * 128, dtype=np.float32),
                           (128, cfg.GG * 128)).astype(BF16).copy()
    ident = np.eye(128, dtype=np.float32)

    nc = build_bass(cfg, meta)

    in_maps = []
    for k in range(cfg.NC):
        own = np.arange(k * cfg.SHARD, (k + 1) * cfg.SHARD)
        other = np.concatenate([np.arange(0, k * cfg.SHARD),
                                np.arange((k + 1) * cfg.SHARD, cfg.N)])
        perm = np.concatenate([own, other])
        xk = np.zeros((cfg.NFEAT, cfg.NA), BF16)
        xk[:, :cfg.N] = xTf[:, perm]
        in_maps.append(dict(
            xT=xk, w1cat=W1cat, w2cat=W2cat, b1rep=b1rep, b2rep=b2rep,
            iota=iota, ident=ident, blob=blob[k], lloc=llocb[k]))

    res = run_bass_kernel_spmd(nc, in_maps, list(range(cfg.NC)), trace=trace)
    outs = [res.results[k]["out"][:cfg.SHARD] for k in range(cfg.NC)]
    full = np.concatenate(outs, axis=0)[:cfg.N].astype(np.float32)
    return full, res


def kernel(**inputs):
    cfg = Cfg()
    out, _ = _run(cfg, inputs, trace=False)
    return out



# revision 18
# speedup vs baseline: 1.6795x; 1.6795x over previous
"""GAT (2-layer, PyG-style) Bass kernel for Trainium2, 8 NeuronCores.

Sharding: 1D destination-node partition. Each core owns N/8 dst nodes; edges
are bucketed by dst so segment-softmax and scatter-add are local. Layer-1
node features (h | a_src | a_dst) are computed redundantly per core into a
per-core-permuted table (own shard first, so dst-side rows fit int16); the
layer-2 table is shard-computed and AllGathered.

Per-edge row fetches use dma_gather (int16 indices, 256B-multiple rows);
src-side indices >= 32768 are handled by a per-group section split over two
table views. Segmented softmax + scatter-add go through a one-hot indicator
matmul on the tensor engine.
"""

import os
import sys

sys.path.insert(0, "/opt/trn_rl_repo")

import numpy as np
import ml_dtypes

BF16 = ml_dtypes.bfloat16

from concourse import bacc, bass, mybir, tile
from concourse.bass_utils import run_bass_kernel_spmd

AF = mybir.ActivationFunctionType
ALU = mybir.AluOpType
DT_BF16 = mybir.dt.bfloat16
DT_F32 = mybir.dt.float32
DT_I16 = mybir.dt.int16
HALF = 32768


class Cfg:
    def __init__(self, N=50000, E=800000):
        self.N = N
        self.E = E
        self.NC = 8
        self.NFEAT = 256
        self.NHID = 16
        self.HEADS = 8
        self.NCLASS = 40
        self.FH = self.HEADS * self.NHID          # 128
        self.T1W = 256                            # table1 row: h|a_src|a_dst|pad
        self.T2W = 128                            # table2 row: h2|a_src2|a_dst2|pad
        assert N % self.NC == 0
        self.SHARD = N // self.NC
        self.NDT = (self.SHARD + 127) // 128      # dst tiles per core
        self.LSH = self.NDT * 128
        self.NROW2 = self.NC * self.LSH
        self.ABATCH = 3
        self.NA = ((N + 384 - 1) // 384) * 384    # stage-A padded rows
        self.GG = 2                               # dst tiles per gather group



CHUNK = 1024


def _gather_chunked(nc, out_tile, col0, table_view, idx_tile, o16, total, elem):
    """Emit dma_gather calls of <=CHUNK idxs; out columns start at col0."""
    done = 0
    while done < total:
        n = min(CHUNK, total - done)
        nc.gpsimd.dma_gather(
            out_tile[:, (col0 + done) // 128:(col0 + done + n) // 128, :],
            table_view,
            idx_tile[:, o16 + done // 16:o16 + (done + n) // 16],
            n, n, elem)
        done += n

def _wrap16(vals):
    """int16 values -> dma_gather wrapped layout [128, len/16]."""
    n = len(vals)
    assert n % 16 == 0
    w = np.asarray(vals, np.int16).reshape(n // 16, 16).T  # [16, n/16]
    return np.tile(w, (8, 1))                              # [128, n/16]


def _prep_edges(cfg, edge_index):
    """Per-core, per-group edge layout with 4 sections by
    (src row>=32768 in table1, src row>=32768 in table2)."""
    N, NC, NDT, SHARD, LSH, GG = (cfg.N, cfg.NC, cfg.NDT, cfg.SHARD,
                                  cfg.LSH, cfg.GG)
    src = np.concatenate([np.asarray(edge_index[0]), np.arange(N)]).astype(np.int64)
    dst = np.concatenate([np.asarray(edge_index[1]), np.arange(N)]).astype(np.int64)
    core = dst // SHARD
    ldst = dst - core * SHARD                      # local dst 0..SHARD-1
    row2 = src + (src // SHARD) * (LSH - SHARD)    # table2 row of src
    groups = [(g, min(g + GG, NDT)) for g in range(0, NDT, GG)]
    NG = len(groups)
    tl = ldst // 128
    gl = np.searchsorted(np.array([a for a, _ in groups]), tl, side="right") - 1

    # per-core src row in the permuted table1: own shard first, others by id
    row1 = np.empty((NC, N), np.int64)
    for k in range(NC):
        own = np.arange(k * SHARD, (k + 1) * SHARD)
        other = np.concatenate([np.arange(0, k * SHARD),
                                np.arange((k + 1) * SHARD, N)])
        perm = np.concatenate([own, other])
        inv = np.empty(N, np.int64)
        inv[perm] = np.arange(N)
        row1[k] = inv

    key = core * NG + gl
    order = np.argsort(key, kind="stable")
    ks = key[order]
    bounds = np.searchsorted(ks, np.arange(NC * NG + 1))

    # section membership per (core, edge): 2*(row1>=H) + (row2>=H)
    # first pass: section counts -> shared section sizes CS[g, s]
    secs = [[None] * NG for _ in range(NC)]
    cnt = np.zeros((NC, NG, 4), np.int64)
    for k in range(NC):
        for g in range(NG):
            ids = order[bounds[k * NG + g]:bounds[k * NG + g + 1]]
            s = 2 * (row1[k][src[ids]] >= HALF) + (row2[ids] >= HALF)
            # order: section, then tile, stable
            o2 = np.lexsort((tl[ids], s))
            ids = ids[o2]
            s = s[o2]
            secs[k][g] = (ids, s)
            cnt[k, g] = np.bincount(s, minlength=4)
    CS = (-(-cnt.max(axis=0) // 128) * 128).astype(np.int64)  # [NG, 4]
    Call = CS.sum(axis=1)                                     # slots per group
    NCOL = (Call // 128).astype(np.int64)

    # blob layout per group (int16 cols): src1lo, src1hi, dst, s2_0..3
    # col counts: (CS0+CS1)/16, (CS2+CS3)/16, Call/16, CS0/16.. CS3/16
    blob_cols = ((CS[:, 0] + CS[:, 1]) + (CS[:, 2] + CS[:, 3])
                 + Call + Call) // 16
    blob_off = np.concatenate([[0], np.cumsum(blob_cols)]).astype(int)
    BLOBTOT = int(blob_off[-1])
    lloc_off = np.concatenate([[0], np.cumsum(NCOL)]).astype(int)
    LLTOT = int(lloc_off[-1])

    blob = np.zeros((NC, 128, BLOBTOT), np.int16)
    llocb = np.full((NC, 128, LLTOT), 1000.0, np.float32)
    lloctb = np.full((NC, LLTOT, 128), 1000.0, np.float32)

    # per (group, tile): column ranges [(c0, c1), ...] in group-local columns
    tile_ranges = [[[] for _ in range(NDT)] for _ in range(NG)]

    for k in range(NC):
        for g, (glo, ghi) in enumerate(groups):
            ids, s = secs[k][g]
            # slot position: section base + within-section position
            sbase = np.concatenate([[0], np.cumsum(CS[g])])[:4]
            pos = np.empty(len(ids), np.int64)
            for sec in range(4):
                m = s == sec
                pos[m] = sbase[sec] + np.arange(m.sum())
            C = int(Call[g])
            # index arrays, padded with 0 (valid row 0)
            r1 = np.zeros(C, np.int64)
            r2v = np.zeros(C, np.int64)
            dl = np.zeros(C, np.int64)
            lv = np.full(C, 1000.0, np.float32)
            r1[pos] = row1[k][src[ids]]
            r2v[pos] = row2[ids]
            dl[pos] = ldst[ids]
            lv[pos] = (ldst[ids] - glo * 128).astype(np.float32)
            c01 = int(CS[g, 0] + CS[g, 1])
            seg = []
            seg.append(_wrap16(r1[:c01]))                        # src1 low
            seg.append(_wrap16(r1[c01:] - HALF * (r1[c01:] >= HALF)))  # src1 hi
            seg.append(_wrap16(dl))                              # dst (both layers)
            cb = np.concatenate([[0], np.cumsum(CS[g])]).astype(int)
            for sec in range(4):
                v = r2v[cb[sec]:cb[sec + 1]]
                seg.append(_wrap16(v - HALF * (v >= HALF)))
            blob[k, :, blob_off[g]:blob_off[g + 1]] = np.concatenate(seg, axis=1)
            llocb[k, :, lloc_off[g]:lloc_off[g + 1]] = \
                lv.reshape(int(NCOL[g]), 128).T
            lloctb[k, lloc_off[g]:lloc_off[g + 1], :] = \
                lv.reshape(int(NCOL[g]), 128)
            if k == 0:
                # column ranges per tile (shared: derive from all cores below)
                pass
    # tile column ranges: union over cores of occupied columns per (g, t)
    occ = np.zeros((NG, NDT, 1), object)
    for g, (glo, ghi) in enumerate(groups):
        ncol = int(NCOL[g])
        used = np.zeros((NDT, ncol), bool)
        for k in range(NC):
            ids, s = secs[k][g]
            sbase = np.concatenate([[0], np.cumsum(CS[g])])[:4]
            pos = np.empty(len(ids), np.int64)
            for sec in range(4):
                m = s == sec
                pos[m] = sbase[sec] + np.arange(m.sum())
            t_of = tl[ids]
            for t in range(glo, ghi):
                cols = np.unique(pos[t_of == t] // 128)
                used[t, cols] = True
        for t in range(glo, ghi):
            cols = np.where(used[t])[0]
            ranges = []
            if len(cols):
                brk = np.where(np.diff(cols) > 1)[0]
                st = 0
                for b in list(brk) + [len(cols) - 1]:
                    ranges.append((int(cols[st]), int(cols[b]) + 1))
                    st = b + 1
            tile_ranges[g][t] = ranges

    meta = dict(groups=groups, CS=CS, Call=Call, NCOL=NCOL,
                blob_off=blob_off, BLOBTOT=BLOBTOT,
                lloc_off=lloc_off, LLTOT=LLTOT, tile_ranges=tile_ranges)
    return meta, blob, llocb.astype(BF16), lloctb.astype(BF16)


def _prep_weights(cfg, W1, att_src1, att_dst1, b1, W2, att_src2, att_dst2, b2):
    W1 = np.asarray(W1, np.float32)
    A1 = np.zeros((cfg.FH, 2 * cfg.HEADS), np.float32)
    for h in range(cfg.HEADS):
        A1[h * cfg.NHID:(h + 1) * cfg.NHID, h] = np.asarray(att_src1)[h]
        A1[h * cfg.NHID:(h + 1) * cfg.NHID, cfg.HEADS + h] = np.asarray(att_dst1)[h]
    W1cat = np.concatenate([W1, W1 @ A1], axis=1).astype(BF16)  # [NFEAT, 144]
    W2cat = np.zeros((cfg.FH, 48), np.float32)
    W2cat[:, :cfg.NCLASS] = np.asarray(W2)
    W2cat[:, cfg.NCLASS] = np.asarray(W2) @ np.asarray(att_src2)[0]
    W2cat[:, cfg.NCLASS + 1] = np.asarray(W2) @ np.asarray(att_dst2)[0]
    W2cat = W2cat.astype(BF16)
    b1rep = np.broadcast_to(np.asarray(b1, np.float32), (128, cfg.FH)).copy()
    b2rep = np.broadcast_to(np.asarray(b2, np.float32), (128, cfg.NCLASS)).copy()
    return W1cat, W2cat, b1rep, b2rep


def build_bass(cfg, meta):
    nc = bacc.Bacc("TRN2", target_bir_lowering=False, debug=False)
    NDT, GG = cfg.NDT, cfg.GG
    TBW = cfg.FH + 2 * cfg.HEADS                   # 144 live cols of table1
    NAB = cfg.NA // (128 * cfg.ABATCH)
    groups = meta["groups"]
    CS, Call, NCOL = meta["CS"], meta["Call"], meta["NCOL"]
    blob_off, lloc_off = meta["blob_off"], meta["lloc_off"]
    tile_ranges = meta["tile_ranges"]
    NCOLMAX = int(max(NCOL))
    BLOBMAX = int(max(blob_off[i + 1] - blob_off[i] for i in range(len(groups))))
    CMAX_T = 1
    for g in range(len(groups)):
        for t in range(NDT):
            if tile_ranges[g][t]:
                CMAX_T = max(CMAX_T,
                             sum(b - a for (a, b) in tile_ranges[g][t]))

    xT = nc.dram_tensor("xT", [cfg.NFEAT, cfg.NA], DT_BF16, kind="ExternalInput")
    w1cat = nc.dram_tensor("w1cat", [cfg.NFEAT, TBW], DT_BF16, kind="ExternalInput")
    w2cat = nc.dram_tensor("w2cat", [cfg.FH, 48], DT_BF16, kind="ExternalInput")
    b1rep_d = nc.dram_tensor("b1rep", [128, cfg.FH], DT_F32, kind="ExternalInput")
    b2rep_d = nc.dram_tensor("b2rep", [128, cfg.NCLASS], DT_F32, kind="ExternalInput")
    iota_d = nc.dram_tensor("iota", [128, GG * 128], DT_BF16, kind="ExternalInput")
    iota2_d = nc.dram_tensor("iota2", [128, GG], DT_BF16, kind="ExternalInput")
    ident_d = nc.dram_tensor("ident", [128, 128], DT_F32, kind="ExternalInput")
    blob_d = nc.dram_tensor("blob", [128, meta["BLOBTOT"]], DT_I16,
                            kind="ExternalInput")
    lloc_d = nc.dram_tensor("lloc", [128, meta["LLTOT"]], DT_BF16,
                            kind="ExternalInput")
    lloct_d = nc.dram_tensor("lloct", [meta["LLTOT"], 128], DT_BF16,
                             kind="ExternalInput")
    out_d = nc.dram_tensor("out", [cfg.LSH, cfg.NCLASS], DT_F32,
                           kind="ExternalOutput")

    table1 = nc.dram_tensor("table1", [cfg.NA, cfg.T1W], DT_BF16)
    h2own_c = nc.dram_tensor("h2own_c", [cfg.LSH, 48], DT_BF16)
    table2s = nc.dram_tensor("table2s", [cfg.NROW2, 48], DT_BF16,
                             addr_space="Shared")
    table2 = nc.dram_tensor("table2", [cfg.NROW2, cfg.T2W], DT_BF16)

    dbg = os.environ.get("GAT_DEBUG_DUMP") == "1"
    if dbg:
        dbg_t1 = nc.dram_tensor("dbg_t1", [cfg.NA, cfg.T1W], DT_BF16,
                                kind="ExternalOutput")
        dbg_t2 = nc.dram_tensor("dbg_t2", [cfg.NROW2, cfg.T2W], DT_BF16,
                                kind="ExternalOutput")
        dbg_ps = nc.dram_tensor("dbg_ps", [cfg.LSH, TBW], DT_F32,
                                kind="ExternalOutput")
        dbg_g1 = nc.dram_tensor("dbg_g1", [128, NCOLMAX, cfg.T1W], DT_BF16,
                                kind="ExternalOutput")

    with tile.TileContext(nc) as tc:
        with tc.tile_pool(name="const", bufs=1) as cpool:
            w1_sb = cpool.tile([128, cfg.NFEAT // 128, TBW], DT_BF16)
            nc.sync.dma_start(out=w1_sb[:],
                              in_=w1cat[:].rearrange("(kt p) c -> p kt c", p=128))
            w2_sb = cpool.tile([128, 48], DT_BF16)
            nc.sync.dma_start(out=w2_sb[:], in_=w2cat[:])
            b1_sb = cpool.tile([128, cfg.FH], DT_F32)
            nc.sync.dma_start(out=b1_sb[:], in_=b1rep_d[:])
            b2_sb = cpool.tile([128, cfg.NCLASS], DT_F32)
            nc.sync.dma_start(out=b2_sb[:], in_=b2rep_d[:])
            iota_sb = cpool.tile([128, GG * 128], DT_BF16)
            nc.sync.dma_start(out=iota_sb[:], in_=iota_d[:])
            iota2_sb = cpool.tile([128, GG], DT_BF16)
            nc.sync.dma_start(out=iota2_sb[:], in_=iota2_d[:])
            ident_sb = cpool.tile([128, 128], DT_F32)
            nc.sync.dma_start(out=ident_sb[:], in_=ident_d[:])
            outf_sb = cpool.tile([128, NDT, cfg.NCLASS], DT_F32)
            adst1_sb = cpool.tile([128, NDT, cfg.HEADS], DT_BF16)
            adst2_sb = cpool.tile([128, NDT, 1], DT_BF16)

            # ============ stage A: table1 + local a_dst table ================
            with (
                tc.tile_pool(name="ax", bufs=3) as axp,
                tc.tile_pool(name="atb", bufs=3) as atbp,
                tc.tile_pool(name="apsum", bufs=2, space="PSUM") as app,
            ):
                for bidx in range(NAB):
                    n0 = bidx * 128 * cfg.ABATCH
                    xt = axp.tile([128, cfg.NFEAT // 128, 128 * cfg.ABATCH],
                                  DT_BF16, tag="xt")
                    for kt in range(cfg.NFEAT // 128):
                        nc.sync.dma_start(
                            out=xt[:, kt, :],
                            in_=xT[kt * 128:(kt + 1) * 128,
                                   n0:n0 + 128 * cfg.ABATCH])
                    pa = app.tile([128, cfg.ABATCH * TBW], DT_F32, tag="pa")
                    for m in range(cfg.ABATCH):
                        for kt in range(cfg.NFEAT // 128):
                            nc.tensor.matmul(
                                out=pa[:, m * TBW:(m + 1) * TBW],
                                lhsT=xt[:, kt, m * 128:(m + 1) * 128],
                                rhs=w1_sb[:, kt, :],
                                start=(kt == 0),
                                stop=(kt == cfg.NFEAT // 128 - 1))
                    tb = atbp.tile([128, cfg.ABATCH * TBW], DT_BF16, tag="tb")
                    nc.vector.tensor_copy(out=tb[:], in_=pa[:])
                    nc.sync.dma_start(
                        out=table1[n0:n0 + 128 * cfg.ABATCH, 0:TBW].rearrange(
                            "(m p) c -> p m c", p=128),
                        in_=tb[:].rearrange("p (m c) -> p m c", c=TBW))
                    # local a_dst rows (a_dst = psum cols FH+8 : FH+16)
                    for m in range(cfg.ABATCH):
                        r0 = n0 + m * 128
                        if r0 >= cfg.LSH:
                            break
                        nc.vector.tensor_copy(
                            out=adst1_sb[:, r0 // 128, :],
                            in_=pa[:, m * TBW + cfg.FH + cfg.HEADS:
                                   m * TBW + cfg.FH + 2 * cfg.HEADS])

            tc.strict_bb_all_engine_barrier()

            # ============ stage B/C: layer-1 edges + layer-2 table ===========
            with (
                tc.tile_pool(name="gx", bufs=2) as gxp,
                tc.tile_pool(name="gi", bufs=2) as gip,
                tc.tile_pool(name="mm", bufs=2) as mmp,
                tc.tile_pool(name="lt", bufs=2) as ltp,
                tc.tile_pool(name="mt", bufs=2) as mtp,
                tc.tile_pool(name="rh", bufs=2) as rhp,
                tc.tile_pool(name="sm", bufs=3) as smp,
                tc.tile_pool(name="cc", bufs=2) as ccp,
                tc.tile_pool(name="ps1", bufs=2, space="PSUM") as ps1,
                tc.tile_pool(name="ps2", bufs=2, space="PSUM") as ps2,
                tc.tile_pool(name="ps3", bufs=2, space="PSUM") as ps3,
                tc.tile_pool(name="pae", bufs=2, space="PSUM") as pae,
            ):
                def load_idx(g):
                    bo = int(blob_off[g])
                    bw = int(blob_off[g + 1]) - bo
                    idx = gip.tile([128, BLOBMAX], DT_I16, tag="idx")
                    nc.sync.dma_start(out=idx[:, :bw], in_=blob_d[:, bo:bo + bw])
                    ll = gip.tile([128, NCOLMAX], DT_BF16, tag="ll")
                    lo = int(lloc_off[g])
                    lw = int(lloc_off[g + 1]) - lo
                    nc.sync.dma_start(out=ll[:, :lw], in_=lloc_d[:, lo:lo + lw])
                    return idx, ll

                def col_tiles_of(g, glo, ghi):
                    """column -> [(tile, local col in that tile's MT)]"""
                    seq = {}
                    for t in range(glo, ghi):
                        cpos = 0
                        for (a, b) in tile_ranges[g][t]:
                            for c in range(a, b):
                                seq.setdefault(c, []).append((t, cpos + c - a))
                            cpos += b - a
                    return seq

                def build_aedge(g, glo, ghi, adst_sb, width):
                    """per-edge a_dst via transposed one-hot matmul:
                    ae[p, c, :] = adst[dst(p, c), :]"""
                    lo = int(lloc_off[g])
                    ncol = int(NCOL[g])
                    ltr = ltp.tile([128, NCOLMAX, 128], DT_BF16, tag="ltr")
                    nc.sync.dma_start(
                        out=ltr[:, :ncol, :],
                        in_=lloct_d[lo:lo + ncol, :].unsqueeze(0)
                            .to_broadcast([128, ncol, 128]))
                    MTs = {}
                    for t in range(glo, ghi):
                        ranges = tile_ranges[g][t]
                        if not ranges:
                            continue
                        MT = mtp.tile([128, max(CMAX_T, 1), 128], DT_BF16,
                                      tag=f"MT{t - glo}")
                        cpos = 0
                        for (a, b) in ranges:
                            w = b - a
                            nc.vector.tensor_tensor(
                                out=MT[:, cpos:cpos + w, :],
                                in0=ltr[:, a:b, :],
                                in1=iota2_sb[:, t - glo:t - glo + 1]
                                    .to_broadcast([128, w, 128]),
                                op=ALU.is_equal)
                            cpos += w
                        MTs[t] = MT
                    ae = pae.tile([128, NCOLMAX, 8], DT_F32, tag="ae")
                    for c, lst in sorted(col_tiles_of(g, glo, ghi).items()):
                        for i, (t, mc) in enumerate(lst):
                            nc.tensor.matmul(
                                out=ae[:, c, 0:width],
                                lhsT=MTs[t][:, mc, :],
                                rhs=adst_sb[:, t, 0:width],
                                start=(i == 0), stop=(i == len(lst) - 1))
                    aesb = smp.tile([128, NCOLMAX, 8], DT_BF16, tag="aesb")
                    nc.vector.tensor_copy(out=aesb[:, :ncol, 0:width],
                                          in_=ae[:, :ncol, 0:width])
                    return aesb

                def build_M(g, t, glo, ll):
                    ranges = tile_ranges[g][t]
                    ncols_t = sum(b - a for (a, b) in ranges)
                    M = mmp.tile([128, max(CMAX_T, 1), 128], DT_BF16, tag="M")
                    cpos = 0
                    cols = []
                    for (a, b) in ranges:
                        w = b - a
                        nc.vector.tensor_tensor(
                            out=M[:, cpos:cpos + w, :],
                            in0=ll[:, a:b].to_broadcast([128, w, 128]),
                            in1=iota_sb[:, (t - glo) * 128:(t - glo + 1) * 128]
                                .unsqueeze(1).to_broadcast([128, w, 128]),
                            op=ALU.is_equal)
                        for c in range(a, b):
                            cols.append((cpos + c - a, c))
                        cpos += w
                    return M, cols, ncols_t

                for g, (glo, ghi) in enumerate(groups):
                    C = int(Call[g])
                    ncol = int(NCOL[g])
                    cs = [int(v) for v in CS[g]]
                    c01 = cs[0] + cs[1]
                    c23 = cs[2] + cs[3]
                    idx, ll = load_idx(g)
                    # offsets into idx blob (cols of 16 idx each)
                    o = 0
                    o_s1lo = o; o += c01 // 16
                    o_s1hi = o; o += c23 // 16
                    o_dst = o; o += C // 16
                    o_s2 = []
                    for sec in range(4):
                        o_s2.append(o); o += cs[sec] // 16
                    g1 = gxp.tile([128, NCOLMAX, cfg.T1W], DT_BF16, tag="g1")
                    if c01:
                        _gather_chunked(nc, g1, 0,
                                        table1[0:min(HALF, cfg.NA), :],
                                        idx, o_s1lo, c01, cfg.T1W)
                    if c23:
                        _gather_chunked(nc, g1, c01, table1[HALF:cfg.NA, :],
                                        idx, o_s1hi, c23, cfg.T1W)
                    aesb = build_aedge(g, glo, ghi, adst1_sb, cfg.HEADS)
                    if dbg and g == 0:
                        nc.sync.dma_start(out=dbg_g1[:, :ncol, :],
                                          in_=g1[:, :ncol, :])
                    # edgewise: logits -> leaky -> exp ; messages
                    lg = smp.tile([128, NCOLMAX, cfg.HEADS], DT_BF16, tag="lg")
                    nc.vector.tensor_tensor(
                        out=lg[:, :ncol, :],
                        in0=g1[:, :ncol, cfg.FH:cfg.FH + cfg.HEADS],
                        in1=aesb[:, :ncol, 0:cfg.HEADS], op=ALU.add)
                    lr = smp.tile([128, NCOLMAX, cfg.HEADS], DT_BF16, tag="lr")
                    nc.vector.tensor_scalar_mul(
                        out=lr[:, :ncol, :], in0=lg[:, :ncol, :], scalar1=0.2)
                    nc.vector.tensor_tensor(
                        out=lr[:, :ncol, :], in0=lr[:, :ncol, :],
                        in1=lg[:, :ncol, :], op=ALU.max)
                    rhs = rhp.tile([128, NCOLMAX, cfg.FH + cfg.HEADS], DT_BF16,
                                   tag="rhs")
                    nc.scalar.activation(
                        out=rhs[:, :ncol, cfg.FH:], in_=lr[:, :ncol, :],
                        func=AF.Exp)
                    for hh in range(cfg.HEADS):
                        nc.vector.tensor_tensor(
                            out=rhs[:, :ncol, hh * cfg.NHID:(hh + 1) * cfg.NHID],
                            in0=g1[:, :ncol, hh * cfg.NHID:(hh + 1) * cfg.NHID],
                            in1=rhs[:, :ncol, cfg.FH + hh:cfg.FH + hh + 1]
                                .to_broadcast([128, ncol, cfg.NHID]),
                            op=ALU.mult)
                    for t in range(glo, ghi):
                        M, cols, nct = build_M(g, t, glo, ll)
                        pseg = ps1.tile([128, TBW], DT_F32, tag="pseg")
                        for i, (mc, c) in enumerate(cols):
                            nc.tensor.matmul(
                                out=pseg[:, 0:cfg.FH + cfg.HEADS],
                                lhsT=M[:, mc, :], rhs=rhs[:, c, :],
                                start=(i == 0), stop=(i == len(cols) - 1))
                        if dbg:
                            psc = ccp.tile([128, TBW], DT_F32, tag="psc")
                            nc.vector.tensor_copy(out=psc[:], in_=pseg[:])
                            nc.sync.dma_start(
                                out=dbg_ps[t * 128:(t + 1) * 128, :], in_=psc[:])
                        # ---- stage C ----
                        rec = ccp.tile([128, cfg.HEADS], DT_F32, tag="rec")
                        nc.vector.reciprocal(
                            out=rec[:], in_=pseg[:, cfg.FH:cfg.FH + cfg.HEADS])
                        o1 = ccp.tile([128, cfg.FH], DT_F32, tag="o1")
                        nc.vector.tensor_tensor(
                            out=o1[:].rearrange("p (h c) -> p h c", c=cfg.NHID),
                            in0=pseg[:, 0:cfg.FH].rearrange(
                                "p (h c) -> p h c", c=cfg.NHID),
                            in1=rec[:].to_broadcast([128, cfg.HEADS, cfg.NHID]),
                            op=ALU.mult)
                        nc.vector.tensor_tensor(
                            out=o1[:], in0=o1[:], in1=b1_sb[:], op=ALU.add)
                        tmin = ccp.tile([128, cfg.FH], DT_F32, tag="tmin")
                        nc.vector.tensor_scalar_min(
                            out=tmin[:], in0=o1[:], scalar1=0.0)
                        nc.scalar.activation(out=tmin[:], in_=tmin[:],
                                             func=AF.Exp)
                        nc.vector.tensor_scalar_add(
                            out=tmin[:], in0=tmin[:], scalar1=-1.0)
                        a1t = ccp.tile([128, cfg.FH], DT_F32, tag="a1t")
                        nc.vector.tensor_tensor(
                            out=a1t[:], in0=o1[:], in1=tmin[:], op=ALU.max)
                        ptr = ps2.tile([128, 128], DT_F32, tag="ptr")
                        nc.tensor.transpose(out=ptr[:], in_=a1t[:],
                                            identity=ident_sb[:])
                        a1T = ccp.tile([128, 128], DT_BF16, tag="a1T")
                        nc.scalar.copy(out=a1T[:], in_=ptr[:])
                        ph2 = ps3.tile([128, 48], DT_F32, tag="ph2")
                        nc.tensor.matmul(out=ph2[:], lhsT=a1T[:], rhs=w2_sb[:],
                                         start=True, stop=True)
                        t2 = ccp.tile([128, 48], DT_BF16, tag="t2")
                        nc.scalar.copy(out=t2[:], in_=ph2[:])
                        nc.sync.dma_start(
                            out=h2own_c[t * 128:(t + 1) * 128, :], in_=t2[:])
                        nc.vector.tensor_copy(
                            out=adst2_sb[:, t, :],
                            in_=t2[:, cfg.NCLASS + 1:cfg.NCLASS + 2])

                # ---- AllGather layer-2 table; widen rows to 128 ----
                nc.gpsimd.collective_compute(
                    "AllGather", ALU.bypass,
                    replica_groups=[list(range(cfg.NC))],
                    ins=[h2own_c[:]], outs=[table2s[:]])
                nc.sync.dma_start(out=table2[:, 0:48], in_=table2s[:])
                tc.strict_bb_all_engine_barrier()

                # ============ stage D/E: layer-2 edges =======================
                NCL = cfg.NCLASS
                for g, (glo, ghi) in enumerate(groups):
                    C = int(Call[g])
                    ncol = int(NCOL[g])
                    cs = [int(v) for v in CS[g]]
                    c01 = cs[0] + cs[1]
                    idx, ll = load_idx(g)
                    o = 0
                    o_s1lo = o; o += c01 // 16
                    o_s1hi = o; o += (cs[2] + cs[3]) // 16
                    o_dst = o; o += C // 16
                    o_s2 = []
                    for sec in range(4):
                        o_s2.append(o); o += cs[sec] // 16
                    g2 = gxp.tile([128, NCOLMAX, cfg.T2W], DT_BF16, tag="gd")
                    cb = 0
                    for sec in range(4):
                        if cs[sec] == 0:
                            continue
                        tv = table2[0:min(HALF, cfg.NROW2), :] if sec in (0, 2) \
                            else table2[HALF:cfg.NROW2, :]
                        _gather_chunked(nc, g2, cb, tv, idx, o_s2[sec],
                                        cs[sec], cfg.T2W)
                        cb += cs[sec]
                    aesb = build_aedge(g, glo, ghi, adst2_sb, 1)
                    lg = smp.tile([128, NCOLMAX, 1], DT_BF16, tag="lg")
                    nc.vector.tensor_tensor(
                        out=lg[:, :ncol, :],
                        in0=g2[:, :ncol, NCL:NCL + 1],
                        in1=aesb[:, :ncol, 0:1], op=ALU.add)
                    lr = smp.tile([128, NCOLMAX, 1], DT_BF16, tag="lr")
                    nc.vector.tensor_scalar_mul(
                        out=lr[:, :ncol, :], in0=lg[:, :ncol, :], scalar1=0.2)
                    nc.vector.tensor_tensor(
                        out=lr[:, :ncol, :], in0=lr[:, :ncol, :],
                        in1=lg[:, :ncol, :], op=ALU.max)
                    rhs = rhp.tile([128, NCOLMAX, NCL + 1], DT_BF16, tag="rhs")
                    nc.scalar.activation(
                        out=rhs[:, :ncol, NCL:], in_=lr[:, :ncol, :],
                        func=AF.Exp)
                    nc.vector.tensor_tensor(
                        out=rhs[:, :ncol, 0:NCL],
                        in0=g2[:, :ncol, 0:NCL],
                        in1=rhs[:, :ncol, NCL:NCL + 1]
                            .to_broadcast([128, ncol, NCL]),
                        op=ALU.mult)
                    for t in range(glo, ghi):
                        M, cols, nct = build_M(g, t, glo, ll)
                        pseg = ps1.tile([128, NCL + 1], DT_F32, tag="pseg")
                        for i, (mc, c) in enumerate(cols):
                            nc.tensor.matmul(
                                out=pseg[:], lhsT=M[:, mc, :], rhs=rhs[:, c, :],
                                start=(i == 0), stop=(i == len(cols) - 1))
                        rec = ccp.tile([128, 1], DT_F32, tag="rec")
                        nc.vector.reciprocal(out=rec[:], in_=pseg[:, NCL:])
                        nc.vector.tensor_tensor(
                            out=outf_sb[:, t, :],
                            in0=pseg[:, 0:NCL],
                            in1=rec[:].to_broadcast([128, NCL]),
                            op=ALU.mult)
                        nc.vector.tensor_tensor(
                            out=outf_sb[:, t, :], in0=outf_sb[:, t, :],
                            in1=b2_sb[:], op=ALU.add)

            # ============ stage F: log_softmax ===========================
            with tc.tile_pool(name="fin", bufs=1) as fpp:
                mx = fpp.tile([128, NDT, 1], DT_F32, tag="mx")
                nc.vector.tensor_reduce(
                    out=mx[:], in_=outf_sb[:], axis=mybir.AxisListType.X,
                    op=ALU.max)
                ex = fpp.tile([128, NDT, cfg.NCLASS], DT_F32, tag="ex")
                nc.vector.tensor_tensor(
                    out=ex[:], in0=outf_sb[:],
                    in1=mx[:].to_broadcast([128, NDT, cfg.NCLASS]),
                    op=ALU.subtract)
                nc.scalar.activation(out=ex[:], in_=ex[:], func=AF.Exp)
                sm = fpp.tile([128, NDT, 1], DT_F32, tag="sm")
                nc.vector.tensor_reduce(
                    out=sm[:], in_=ex[:], axis=mybir.AxisListType.X,
                    op=ALU.add)
                nc.scalar.activation(out=sm[:], in_=sm[:], func=AF.Ln)
                nc.vector.tensor_tensor(
                    out=sm[:], in0=sm[:], in1=mx[:], op=ALU.add)
                nc.vector.tensor_tensor(
                    out=outf_sb[:], in0=outf_sb[:],
                    in1=sm[:].to_broadcast([128, NDT, cfg.NCLASS]),
                    op=ALU.subtract)
                nc.sync.dma_start(
                    out=out_d[:].rearrange("(t p) c -> p t c", p=128),
                    in_=outf_sb[:])
                if dbg:
                    nc.sync.dma_start(out=dbg_t1[:], in_=table1[:])
                    nc.sync.dma_start(out=dbg_t2[:], in_=table2[:])
    nc.compile()
    return nc


def _run(cfg, inputs, trace=False):
    meta, blob, llocb, lloctb = _prep_edges(cfg, np.asarray(inputs["edge_index"]))
    W1cat, W2cat, b1rep, b2rep = _prep_weights(
        cfg, inputs["W1"], inputs["att_src1"], inputs["att_dst1"], inputs["b1"],
        inputs["W2"], inputs["att_src2"], inputs["att_dst2"], inputs["b2"])
    x = np.asarray(inputs["x"], np.float32)
    xTf = np.zeros((cfg.NFEAT, cfg.NA), BF16)
    xTf[:, :cfg.N] = x.T.astype(BF16)
    iota = np.broadcast_to(np.arange(cfg.GG * 128, dtype=np.float32),
                           (128, cfg.GG * 128)).astype(BF16).copy()
    iota2 = (np.arange(128, dtype=np.float32)[:, None]
             + 128.0 * np.arange(cfg.GG, dtype=np.float32)[None, :]
             ).astype(BF16).copy()
    ident = np.eye(128, dtype=np.float32)

    nc = build_bass(cfg, meta)

    in_maps = []
    for k in range(cfg.NC):
        own = np.arange(k * cfg.SHARD, (k + 1) * cfg.SHARD)
        other = np.concatenate([np.arange(0, k * cfg.SHARD),
                                np.arange((k + 1) * cfg.SHARD, cfg.N)])
        perm = np.concatenate([own, other])
        xk = np.zeros((cfg.NFEAT, cfg.NA), BF16)
        xk[:, :cfg.N] = xTf[:, perm]
        in_maps.append(dict(
            xT=xk, w1cat=W1cat, w2cat=W2cat, b1rep=b1rep, b2rep=b2rep,
            iota=iota, iota2=iota2, ident=ident, blob=blob[k],
            lloc=llocb[k], lloct=lloctb[k]))

    res = run_bass_kernel_spmd(nc, in_maps, list(range(cfg.NC)), trace=trace)
    outs = [res.results[k]["out"][:cfg.SHARD] for k in range(cfg.NC)]
    full = np.concatenate(outs, axis=0)[:cfg.N].astype(np.float32)
    return full, res


def kernel(**inputs):
    cfg = Cfg()
    out, _ = _run(cfg, inputs, trace=False)
    return out



# revision 22
# speedup vs baseline: 1.7544x; 1.0446x over previous
"""GAT (2-layer, PyG-style) Bass kernel for Trainium2, 8 NeuronCores.

Sharding: 1D destination-node partition. Each core owns N/8 dst nodes; edges
are bucketed by dst so segment-softmax and scatter-add are local. Layer-1
node features (h | a_src | a_dst) are computed redundantly per core into a
per-core-permuted table (own shard first, so dst-side rows fit int16); the
layer-2 table is shard-computed and AllGathered.

Per-edge row fetches use dma_gather (int16 indices, 256B-multiple rows);
src-side indices >= 32768 are handled by a per-group section split over two
table views. Segmented softmax + scatter-add go through a one-hot indicator
matmul on the tensor engine.
"""

import os
import sys

sys.path.insert(0, "/opt/trn_rl_repo")

import numpy as np
import ml_dtypes

BF16 = ml_dtypes.bfloat16

from concourse import bacc, bass, mybir, tile
from concourse.bass_utils import run_bass_kernel_spmd

AF = mybir.ActivationFunctionType
ALU = mybir.AluOpType
DT_BF16 = mybir.dt.bfloat16
DT_F32 = mybir.dt.float32
DT_I16 = mybir.dt.int16
HALF = 32768


class Cfg:
    def __init__(self, N=50000, E=800000):
        self.N = N
        self.E = E
        self.NC = 8
        self.NFEAT = 256
        self.NHID = 16
        self.HEADS = 8
        self.NCLASS = 40
        self.FH = self.HEADS * self.NHID          # 128
        self.T1W = 256                            # table1 row: h|a_src|a_dst|pad
        self.T2W = 128                            # table2 row: h2|a_src2|a_dst2|pad
        assert N % self.NC == 0
        self.SHARD = N // self.NC
        self.NDT = (self.SHARD + 127) // 128      # dst tiles per core
        self.LSH = self.NDT * 128
        self.NROW2 = self.NC * self.LSH
        self.ABATCH = 3
        self.NA = ((N + 384 - 1) // 384) * 384    # stage-A padded rows
        self.GG = 2                               # dst tiles per gather group



CHUNK = 1024


def _gather_chunked(nc, out_tile, col0, table_view, idx_tile, o16, total, elem):
    """Emit dma_gather calls of <=CHUNK idxs; out columns start at col0."""
    done = 0
    while done < total:
        n = min(CHUNK, total - done)
        nc.gpsimd.dma_gather(
            out_tile[:, (col0 + done) // 128:(col0 + done + n) // 128, :],
            table_view,
            idx_tile[:, o16 + done // 16:o16 + (done + n) // 16],
            n, n, elem)
        done += n

def _wrap16(vals):
    """int16 values -> dma_gather wrapped layout [128, len/16]."""
    n = len(vals)
    assert n % 16 == 0
    w = np.asarray(vals, np.int16).reshape(n // 16, 16).T  # [16, n/16]
    return np.tile(w, (8, 1))                              # [128, n/16]


def _prep_edges(cfg, edge_index):
    """Per-core, per-group edge layout with 4 sections by
    (src row>=32768 in table1, src row>=32768 in table2)."""
    N, NC, NDT, SHARD, LSH, GG = (cfg.N, cfg.NC, cfg.NDT, cfg.SHARD,
                                  cfg.LSH, cfg.GG)
    src = np.concatenate([np.asarray(edge_index[0]), np.arange(N)]).astype(np.int64)
    dst = np.concatenate([np.asarray(edge_index[1]), np.arange(N)]).astype(np.int64)
    core = dst // SHARD
    ldst = dst - core * SHARD                      # local dst 0..SHARD-1
    row2 = src + (src // SHARD) * (LSH - SHARD)    # table2 row of src
    groups = [(g, min(g + GG, NDT)) for g in range(0, NDT, GG)]
    NG = len(groups)
    tl = ldst // 128
    gl = np.searchsorted(np.array([a for a, _ in groups]), tl, side="right") - 1

    # per-core src row in the permuted table1: own shard first, others by id
    row1 = np.empty((NC, N), np.int64)
    for k in range(NC):
        own = np.arange(k * SHARD, (k + 1) * SHARD)
        other = np.concatenate([np.arange(0, k * SHARD),
                                np.arange((k + 1) * SHARD, N)])
        perm = np.concatenate([own, other])
        inv = np.empty(N, np.int64)
        inv[perm] = np.arange(N)
        row1[k] = inv

    key = core * NG + gl
    order = np.argsort(key, kind="stable")
    ks = key[order]
    bounds = np.searchsorted(ks, np.arange(NC * NG + 1))

    # section membership per (core, edge): 2*(row1>=H) + (row2>=H)
    # first pass: section counts -> shared section sizes CS[g, s]
    secs = [[None] * NG for _ in range(NC)]
    cnt = np.zeros((NC, NG, 4), np.int64)
    for k in range(NC):
        for g in range(NG):
            ids = order[bounds[k * NG + g]:bounds[k * NG + g + 1]]
            s = 2 * (row1[k][src[ids]] >= HALF) + (row2[ids] >= HALF)
            # order: section, then tile, stable
            o2 = np.lexsort((tl[ids], s))
            ids = ids[o2]
            s = s[o2]
            secs[k][g] = (ids, s)
            cnt[k, g] = np.bincount(s, minlength=4)
    CS = (-(-cnt.max(axis=0) // 128) * 128).astype(np.int64)  # [NG, 4]
    Call = CS.sum(axis=1)                                     # slots per group
    NCOL = (Call // 128).astype(np.int64)

    # blob layout per group (int16 cols): src1lo, src1hi, dst, s2_0..3
    # col counts: (CS0+CS1)/16, (CS2+CS3)/16, Call/16, CS0/16.. CS3/16
    blob_cols = ((CS[:, 0] + CS[:, 1]) + (CS[:, 2] + CS[:, 3])
                 + Call + Call) // 16
    blob_off = np.concatenate([[0], np.cumsum(blob_cols)]).astype(int)
    BLOBTOT = int(blob_off[-1])
    lloc_off = np.concatenate([[0], np.cumsum(NCOL)]).astype(int)
    LLTOT = int(lloc_off[-1])

    blob = np.zeros((NC, 128, BLOBTOT), np.int16)
    llocb = np.full((NC, 128, LLTOT), 1000.0, np.float32)
    lloctb = np.full((NC, LLTOT, 128), 1000.0, np.float32)

    # per (group, tile): column ranges [(c0, c1), ...] in group-local columns
    tile_ranges = [[[] for _ in range(NDT)] for _ in range(NG)]

    for k in range(NC):
        for g, (glo, ghi) in enumerate(groups):
            ids, s = secs[k][g]
            # slot position: section base + within-section position
            sbase = np.concatenate([[0], np.cumsum(CS[g])])[:4]
            pos = np.empty(len(ids), np.int64)
            for sec in range(4):
                m = s == sec
                pos[m] = sbase[sec] + np.arange(m.sum())
            C = int(Call[g])
            # index arrays, padded with 0 (valid row 0)
            r1 = np.zeros(C, np.int64)
            r2v = np.zeros(C, np.int64)
            dl = np.zeros(C, np.int64)
            lv = np.full(C, 1000.0, np.float32)
            r1[pos] = row1[k][src[ids]]
            r2v[pos] = row2[ids]
            dl[pos] = ldst[ids]
            lv[pos] = (ldst[ids] - glo * 128).astype(np.float32)
            c01 = int(CS[g, 0] + CS[g, 1])
            seg = []
            seg.append(_wrap16(r1[:c01]))                        # src1 low
            seg.append(_wrap16(r1[c01:] - HALF * (r1[c01:] >= HALF)))  # src1 hi
            seg.append(_wrap16(dl))                              # dst (both layers)
            cb = np.concatenate([[0], np.cumsum(CS[g])]).astype(int)
            for sec in range(4):
                v = r2v[cb[sec]:cb[sec + 1]]
                seg.append(_wrap16(v - HALF * (v >= HALF)))
            blob[k, :, blob_off[g]:blob_off[g + 1]] = np.concatenate(seg, axis=1)
            llocb[k, :, lloc_off[g]:lloc_off[g + 1]] = \
                lv.reshape(int(NCOL[g]), 128).T
            lloctb[k, lloc_off[g]:lloc_off[g + 1], :] = \
                lv.reshape(int(NCOL[g]), 128)
            if k == 0:
                # column ranges per tile (shared: derive from all cores below)
                pass
    # tile column ranges: union over cores of occupied columns per (g, t)
    occ = np.zeros((NG, NDT, 1), object)
    for g, (glo, ghi) in enumerate(groups):
        ncol = int(NCOL[g])
        used = np.zeros((NDT, ncol), bool)
        for k in range(NC):
            ids, s = secs[k][g]
            sbase = np.concatenate([[0], np.cumsum(CS[g])])[:4]
            pos = np.empty(len(ids), np.int64)
            for sec in range(4):
                m = s == sec
                pos[m] = sbase[sec] + np.arange(m.sum())
            t_of = tl[ids]
            for t in range(glo, ghi):
                cols = np.unique(pos[t_of == t] // 128)
                used[t, cols] = True
        for t in range(glo, ghi):
            cols = np.where(used[t])[0]
            ranges = []
            if len(cols):
                brk = np.where(np.diff(cols) > 1)[0]
                st = 0
                for b in list(brk) + [len(cols) - 1]:
                    ranges.append((int(cols[st]), int(cols[b]) + 1))
                    st = b + 1
            tile_ranges[g][t] = ranges

    meta = dict(groups=groups, CS=CS, Call=Call, NCOL=NCOL,
                blob_off=blob_off, BLOBTOT=BLOBTOT,
                lloc_off=lloc_off, LLTOT=LLTOT, tile_ranges=tile_ranges)
    return meta, blob, llocb.astype(BF16), lloctb.astype(BF16)


def _prep_weights(cfg, W1, att_src1, att_dst1, b1, W2, att_src2, att_dst2, b2):
    W1 = np.asarray(W1, np.float32)
    A1 = np.zeros((cfg.FH, 2 * cfg.HEADS), np.float32)
    for h in range(cfg.HEADS):
        A1[h * cfg.NHID:(h + 1) * cfg.NHID, h] = np.asarray(att_src1)[h]
        A1[h * cfg.NHID:(h + 1) * cfg.NHID, cfg.HEADS + h] = np.asarray(att_dst1)[h]
    W1cat = np.concatenate([W1, W1 @ A1], axis=1).astype(BF16)  # [NFEAT, 144]
    W2cat = np.zeros((cfg.FH, 48), np.float32)
    W2cat[:, :cfg.NCLASS] = np.asarray(W2)
    W2cat[:, cfg.NCLASS] = np.asarray(W2) @ np.asarray(att_src2)[0]
    W2cat[:, cfg.NCLASS + 1] = np.asarray(W2) @ np.asarray(att_dst2)[0]
    W2cat = W2cat.astype(BF16)
    b1rep = np.broadcast_to(np.asarray(b1, np.float32), (128, cfg.FH)).copy()
    b2rep = np.broadcast_to(np.asarray(b2, np.float32), (128, cfg.NCLASS)).copy()
    return W1cat, W2cat, b1rep, b2rep


def build_bass(cfg, meta):
    nc = bacc.Bacc("TRN2", target_bir_lowering=False, debug=False)
    NDT, GG = cfg.NDT, cfg.GG
    TBW = cfg.FH + 2 * cfg.HEADS                   # 144 live cols of table1
    NAB = cfg.NA // (128 * cfg.ABATCH)
    groups = meta["groups"]
    CS, Call, NCOL = meta["CS"], meta["Call"], meta["NCOL"]
    blob_off, lloc_off = meta["blob_off"], meta["lloc_off"]
    tile_ranges = meta["tile_ranges"]
    NCOLMAX = int(max(NCOL))
    BLOBMAX = int(max(blob_off[i + 1] - blob_off[i] for i in range(len(groups))))
    CMAX_T = 1
    for g in range(len(groups)):
        for t in range(NDT):
            if tile_ranges[g][t]:
                CMAX_T = max(CMAX_T,
                             sum(b - a for (a, b) in tile_ranges[g][t]))

    xT = nc.dram_tensor("xT", [cfg.NFEAT, cfg.NA], DT_BF16, kind="ExternalInput")
    w1cat = nc.dram_tensor("w1cat", [cfg.NFEAT, TBW], DT_BF16, kind="ExternalInput")
    w2cat = nc.dram_tensor("w2cat", [cfg.FH, 48], DT_BF16, kind="ExternalInput")
    b1rep_d = nc.dram_tensor("b1rep", [128, cfg.FH], DT_F32, kind="ExternalInput")
    b2rep_d = nc.dram_tensor("b2rep", [128, cfg.NCLASS], DT_F32, kind="ExternalInput")
    iota_d = nc.dram_tensor("iota", [128, GG * 128], DT_BF16, kind="ExternalInput")
    iota2_d = nc.dram_tensor("iota2", [128, GG], DT_BF16, kind="ExternalInput")
    ident_d = nc.dram_tensor("ident", [128, 128], DT_F32, kind="ExternalInput")
    blob_d = nc.dram_tensor("blob", [128, meta["BLOBTOT"]], DT_I16,
                            kind="ExternalInput")
    lloc_d = nc.dram_tensor("lloc", [128, meta["LLTOT"]], DT_BF16,
                            kind="ExternalInput")
    lloct_d = nc.dram_tensor("lloct", [meta["LLTOT"], 128], DT_BF16,
                             kind="ExternalInput")
    out_d = nc.dram_tensor("out", [cfg.LSH, cfg.NCLASS], DT_F32,
                           kind="ExternalOutput")

    table1 = nc.dram_tensor("table1", [cfg.NA, cfg.T1W], DT_BF16)
    h2own_c = nc.dram_tensor("h2own_c", [cfg.LSH, 48], DT_BF16)
    # stage-B-built one-hot matrices, reloaded in stage D instead of rebuilt
    moff = {}
    off = 0
    for g in range(len(groups)):
        for t in range(NDT):
            w = sum(b - a for (a, b) in tile_ranges[g][t])
            if w:
                moff[(g, t)] = off
                off += w
    MTOT = max(off, 1)
    m_dram = nc.dram_tensor("m_dram", [128, MTOT, 128], DT_BF16)
    mt_dram = nc.dram_tensor("mt_dram", [128, MTOT, 128], DT_BF16)
    table2s = nc.dram_tensor("table2s", [cfg.NROW2, 48], DT_BF16,
                             addr_space="Shared")
    table2 = nc.dram_tensor("table2", [cfg.NROW2, cfg.T2W], DT_BF16)

    dbg = os.environ.get("GAT_DEBUG_DUMP") == "1"
    if dbg:
        dbg_t1 = nc.dram_tensor("dbg_t1", [cfg.NA, cfg.T1W], DT_BF16,
                                kind="ExternalOutput")
        dbg_t2 = nc.dram_tensor("dbg_t2", [cfg.NROW2, cfg.T2W], DT_BF16,
                                kind="ExternalOutput")
        dbg_ps = nc.dram_tensor("dbg_ps", [cfg.LSH, TBW], DT_F32,
                                kind="ExternalOutput")
        dbg_g1 = nc.dram_tensor("dbg_g1", [128, NCOLMAX, cfg.T1W], DT_BF16,
                                kind="ExternalOutput")

    with tile.TileContext(nc) as tc:
        with tc.tile_pool(name="const", bufs=1) as cpool:
            w1_sb = cpool.tile([128, cfg.NFEAT // 128, TBW], DT_BF16)
            nc.sync.dma_start(out=w1_sb[:],
                              in_=w1cat[:].rearrange("(kt p) c -> p kt c", p=128))
            w2_sb = cpool.tile([128, 48], DT_BF16)
            nc.sync.dma_start(out=w2_sb[:], in_=w2cat[:])
            b1_sb = cpool.tile([128, cfg.FH], DT_F32)
            nc.sync.dma_start(out=b1_sb[:], in_=b1rep_d[:])
            b2_sb = cpool.tile([128, cfg.NCLASS], DT_F32)
            nc.sync.dma_start(out=b2_sb[:], in_=b2rep_d[:])
            iota_sb = cpool.tile([128, GG * 128], DT_BF16)
            nc.sync.dma_start(out=iota_sb[:], in_=iota_d[:])
            iota2_sb = cpool.tile([128, GG], DT_BF16)
            nc.sync.dma_start(out=iota2_sb[:], in_=iota2_d[:])
            ident_sb = cpool.tile([128, 128], DT_F32)
            nc.sync.dma_start(out=ident_sb[:], in_=ident_d[:])
            outf_sb = cpool.tile([128, NDT, cfg.NCLASS], DT_F32)
            adst1_sb = cpool.tile([128, NDT, cfg.HEADS], DT_BF16)
            adst2_sb = cpool.tile([128, NDT, 1], DT_BF16)

            # ============ stage A: table1 + local a_dst table ================
            with (
                tc.tile_pool(name="ax", bufs=3) as axp,
                tc.tile_pool(name="atb", bufs=3) as atbp,
                tc.tile_pool(name="apsum", bufs=2, space="PSUM") as app,
            ):
                for bidx in range(NAB):
                    n0 = bidx * 128 * cfg.ABATCH
                    xt = axp.tile([128, cfg.NFEAT // 128, 128 * cfg.ABATCH],
                                  DT_BF16, tag="xt")
                    for kt in range(cfg.NFEAT // 128):
                        nc.sync.dma_start(
                            out=xt[:, kt, :],
                            in_=xT[kt * 128:(kt + 1) * 128,
                                   n0:n0 + 128 * cfg.ABATCH])
                    pa = app.tile([128, cfg.ABATCH * TBW], DT_F32, tag="pa")
                    for m in range(cfg.ABATCH):
                        for kt in range(cfg.NFEAT // 128):
                            nc.tensor.matmul(
                                out=pa[:, m * TBW:(m + 1) * TBW],
                                lhsT=xt[:, kt, m * 128:(m + 1) * 128],
                                rhs=w1_sb[:, kt, :],
                                start=(kt == 0),
                                stop=(kt == cfg.NFEAT // 128 - 1))
                    tb = atbp.tile([128, cfg.ABATCH * TBW], DT_BF16, tag="tb")
                    nc.scalar.copy(out=tb[:], in_=pa[:])
                    nc.sync.dma_start(
                        out=table1[n0:n0 + 128 * cfg.ABATCH, 0:TBW].rearrange(
                            "(m p) c -> p m c", p=128),
                        in_=tb[:].rearrange("p (m c) -> p m c", c=TBW))
                    # local a_dst rows (a_dst = psum cols FH+8 : FH+16)
                    for m in range(cfg.ABATCH):
                        r0 = n0 + m * 128
                        if r0 >= cfg.LSH:
                            break
                        nc.vector.tensor_copy(
                            out=adst1_sb[:, r0 // 128, :],
                            in_=pa[:, m * TBW + cfg.FH + cfg.HEADS:
                                   m * TBW + cfg.FH + 2 * cfg.HEADS])

            tc.strict_bb_all_engine_barrier()

            # ============ stage B/C: layer-1 edges + layer-2 table ===========
            with (
                tc.tile_pool(name="gx", bufs=2) as gxp,
                tc.tile_pool(name="gi", bufs=2) as gip,
                tc.tile_pool(name="mm", bufs=2) as mmp,
                tc.tile_pool(name="lt", bufs=2) as ltp,
                tc.tile_pool(name="mt", bufs=2) as mtp,
                tc.tile_pool(name="rh", bufs=2) as rhp,
                tc.tile_pool(name="sm", bufs=3) as smp,
                tc.tile_pool(name="cc", bufs=2) as ccp,
                tc.tile_pool(name="ps1", bufs=2, space="PSUM") as ps1,
                tc.tile_pool(name="ps2", bufs=2, space="PSUM") as ps2,
                tc.tile_pool(name="ps3", bufs=2, space="PSUM") as ps3,
                tc.tile_pool(name="pae", bufs=2, space="PSUM") as pae,
            ):
                def load_idx(g):
                    bo = int(blob_off[g])
                    bw = int(blob_off[g + 1]) - bo
                    idx = gip.tile([128, BLOBMAX], DT_I16, tag="idx")
                    nc.sync.dma_start(out=idx[:, :bw], in_=blob_d[:, bo:bo + bw])
                    ll = gip.tile([128, NCOLMAX], DT_BF16, tag="ll")
                    lo = int(lloc_off[g])
                    lw = int(lloc_off[g + 1]) - lo
                    nc.sync.dma_start(out=ll[:, :lw], in_=lloc_d[:, lo:lo + lw])
                    return idx, ll

                def col_tiles_of(g, glo, ghi):
                    """column -> [(tile, local col in that tile's MT)]"""
                    seq = {}
                    for t in range(glo, ghi):
                        cpos = 0
                        for (a, b) in tile_ranges[g][t]:
                            for c in range(a, b):
                                seq.setdefault(c, []).append((t, cpos + c - a))
                            cpos += b - a
                    return seq

                def build_aedge(g, glo, ghi, adst_sb, width, save):
                    """per-edge a_dst via transposed one-hot matmul:
                    ae[p, c, :] = adst[dst(p, c), :]"""
                    lo = int(lloc_off[g])
                    ncol = int(NCOL[g])
                    if save:
                        ltr = ltp.tile([128, NCOLMAX, 128], DT_BF16, tag="ltr")
                        nc.sync.dma_start(
                            out=ltr[:, :ncol, :],
                            in_=lloct_d[lo:lo + ncol, :].unsqueeze(0)
                                .to_broadcast([128, ncol, 128]))
                    MTs = {}
                    for t in range(glo, ghi):
                        ranges = tile_ranges[g][t]
                        if not ranges:
                            continue
                        nct = sum(b - a for (a, b) in ranges)
                        MT = mtp.tile([128, max(CMAX_T, 1), 128], DT_BF16,
                                      tag=f"MT{t - glo}")
                        if save:
                            cpos = 0
                            for (a, b) in ranges:
                                w = b - a
                                nc.vector.tensor_tensor(
                                    out=MT[:, cpos:cpos + w, :],
                                    in0=ltr[:, a:b, :],
                                    in1=iota2_sb[:, t - glo:t - glo + 1]
                                        .to_broadcast([128, w, 128]),
                                    op=ALU.is_equal)
                                cpos += w
                            nc.sync.dma_start(
                                out=mt_dram[:, moff[(g, t)]:moff[(g, t)] + nct,
                                            :],
                                in_=MT[:, :nct, :])
                        else:
                            nc.sync.dma_start(
                                out=MT[:, :nct, :],
                                in_=mt_dram[:, moff[(g, t)]:moff[(g, t)] + nct,
                                            :])
                        MTs[t] = MT
                    ae = pae.tile([128, NCOLMAX, 8], DT_F32, tag="ae")
                    for c, lst in sorted(col_tiles_of(g, glo, ghi).items()):
                        for i, (t, mc) in enumerate(lst):
                            nc.tensor.matmul(
                                out=ae[:, c, 0:width],
                                lhsT=MTs[t][:, mc, :],
                                rhs=adst_sb[:, t, 0:width],
                                start=(i == 0), stop=(i == len(lst) - 1))
                    aesb = smp.tile([128, NCOLMAX, 8], DT_BF16, tag="aesb")
                    nc.scalar.copy(out=aesb[:, :ncol, 0:width],
                                   in_=ae[:, :ncol, 0:width])
                    return aesb

                def build_M(g, t, glo, ll, save):
                    ranges = tile_ranges[g][t]
                    ncols_t = sum(b - a for (a, b) in ranges)
                    M = mmp.tile([128, max(CMAX_T, 1), 128], DT_BF16, tag="M")
                    cpos = 0
                    cols = []
                    for (a, b) in ranges:
                        w = b - a
                        if save:
                            nc.vector.tensor_tensor(
                                out=M[:, cpos:cpos + w, :],
                                in0=ll[:, a:b].to_broadcast([128, w, 128]),
                                in1=iota_sb[:,
                                            (t - glo) * 128:(t - glo + 1) * 128]
                                    .unsqueeze(1).to_broadcast([128, w, 128]),
                                op=ALU.is_equal)
                        for c in range(a, b):
                            cols.append((cpos + c - a, c))
                        cpos += w
                    if ncols_t:
                        if save:
                            nc.sync.dma_start(
                                out=m_dram[:, moff[(g, t)]:moff[(g, t)]
                                           + ncols_t, :],
                                in_=M[:, :ncols_t, :])
                        else:
                            nc.sync.dma_start(
                                out=M[:, :ncols_t, :],
                                in_=m_dram[:, moff[(g, t)]:moff[(g, t)]
                                           + ncols_t, :])
                    return M, cols, ncols_t

                for g, (glo, ghi) in enumerate(groups):
                    C = int(Call[g])
                    ncol = int(NCOL[g])
                    cs = [int(v) for v in CS[g]]
                    c01 = cs[0] + cs[1]
                    c23 = cs[2] + cs[3]
                    idx, ll = load_idx(g)
                    # offsets into idx blob (cols of 16 idx each)
                    o = 0
                    o_s1lo = o; o += c01 // 16
                    o_s1hi = o; o += c23 // 16
                    o_dst = o; o += C // 16
                    o_s2 = []
                    for sec in range(4):
                        o_s2.append(o); o += cs[sec] // 16
                    g1 = gxp.tile([128, NCOLMAX, cfg.T1W], DT_BF16, tag="g1")
                    if c01:
                        _gather_chunked(nc, g1, 0,
                                        table1[0:min(HALF, cfg.NA), :],
                                        idx, o_s1lo, c01, cfg.T1W)
                    if c23:
                        _gather_chunked(nc, g1, c01, table1[HALF:cfg.NA, :],
                                        idx, o_s1hi, c23, cfg.T1W)
                    aesb = build_aedge(g, glo, ghi, adst1_sb, cfg.HEADS, True)
                    if dbg and g == 0:
                        nc.sync.dma_start(out=dbg_g1[:, :ncol, :],
                                          in_=g1[:, :ncol, :])
                    # edgewise: logits -> leaky -> exp ; messages
                    lg = smp.tile([128, NCOLMAX, cfg.HEADS], DT_BF16, tag="lg")
                    nc.vector.tensor_tensor(
                        out=lg[:, :ncol, :],
                        in0=g1[:, :ncol, cfg.FH:cfg.FH + cfg.HEADS],
                        in1=aesb[:, :ncol, 0:cfg.HEADS], op=ALU.add)
                    lr = smp.tile([128, NCOLMAX, cfg.HEADS], DT_BF16, tag="lr")
                    nc.vector.tensor_scalar_mul(
                        out=lr[:, :ncol, :], in0=lg[:, :ncol, :], scalar1=0.2)
                    nc.vector.tensor_tensor(
                        out=lr[:, :ncol, :], in0=lr[:, :ncol, :],
                        in1=lg[:, :ncol, :], op=ALU.max)
                    rhs = rhp.tile([128, NCOLMAX, cfg.FH + cfg.HEADS], DT_BF16,
                                   tag="rhs")
                    nc.scalar.activation(
                        out=rhs[:, :ncol, cfg.FH:], in_=lr[:, :ncol, :],
                        func=AF.Exp)
                    for hh in range(cfg.HEADS):
                        nc.vector.tensor_tensor(
                            out=rhs[:, :ncol, hh * cfg.NHID:(hh + 1) * cfg.NHID],
                            in0=g1[:, :ncol, hh * cfg.NHID:(hh + 1) * cfg.NHID],
                            in1=rhs[:, :ncol, cfg.FH + hh:cfg.FH + hh + 1]
                                .to_broadcast([128, ncol, cfg.NHID]),
                            op=ALU.mult)
                    for t in range(glo, ghi):
                        M, cols, nct = build_M(g, t, glo, ll, True)
                        pseg = ps1.tile([128, TBW], DT_F32, tag="pseg")
                        for i, (mc, c) in enumerate(cols):
                            nc.tensor.matmul(
                                out=pseg[:, 0:cfg.FH + cfg.HEADS],
                                lhsT=M[:, mc, :], rhs=rhs[:, c, :],
                                start=(i == 0), stop=(i == len(cols) - 1))
                        if dbg:
                            psc = ccp.tile([128, TBW], DT_F32, tag="psc")
                            nc.vector.tensor_copy(out=psc[:], in_=pseg[:])
                            nc.sync.dma_start(
                                out=dbg_ps[t * 128:(t + 1) * 128, :], in_=psc[:])
                        # ---- stage C ----
                        rec = ccp.tile([128, cfg.HEADS], DT_F32, tag="rec")
                        nc.vector.reciprocal(
                            out=rec[:], in_=pseg[:, cfg.FH:cfg.FH + cfg.HEADS])
                        o1 = ccp.tile([128, cfg.FH], DT_F32, tag="o1")
                        nc.vector.tensor_tensor(
                            out=o1[:].rearrange("p (h c) -> p h c", c=cfg.NHID),
                            in0=pseg[:, 0:cfg.FH].rearrange(
                                "p (h c) -> p h c", c=cfg.NHID),
                            in1=rec[:].to_broadcast([128, cfg.HEADS, cfg.NHID]),
                            op=ALU.mult)
                        nc.vector.tensor_tensor(
                            out=o1[:], in0=o1[:], in1=b1_sb[:], op=ALU.add)
                        tmin = ccp.tile([128, cfg.FH], DT_F32, tag="tmin")
                        nc.vector.tensor_scalar_min(
                            out=tmin[:], in0=o1[:], scalar1=0.0)
                        nc.scalar.activation(out=tmin[:], in_=tmin[:],
                                             func=AF.Exp)
                        nc.vector.tensor_scalar_add(
                            out=tmin[:], in0=tmin[:], scalar1=-1.0)
                        a1t = ccp.tile([128, cfg.FH], DT_F32, tag="a1t")
                        nc.vector.tensor_tensor(
                            out=a1t[:], in0=o1[:], in1=tmin[:], op=ALU.max)
                        ptr = ps2.tile([128, 128], DT_F32, tag="ptr")
                        nc.tensor.transpose(out=ptr[:], in_=a1t[:],
                                            identity=ident_sb[:])
                        a1T = ccp.tile([128, 128], DT_BF16, tag="a1T")
                        nc.scalar.copy(out=a1T[:], in_=ptr[:])
                        ph2 = ps3.tile([128, 48], DT_F32, tag="ph2")
                        nc.tensor.matmul(out=ph2[:], lhsT=a1T[:], rhs=w2_sb[:],
                                         start=True, stop=True)
                        t2 = ccp.tile([128, 48], DT_BF16, tag="t2")
                        nc.scalar.copy(out=t2[:], in_=ph2[:])
                        nc.sync.dma_start(
                            out=h2own_c[t * 128:(t + 1) * 128, :], in_=t2[:])
                        nc.vector.tensor_copy(
                            out=adst2_sb[:, t, :],
                            in_=t2[:, cfg.NCLASS + 1:cfg.NCLASS + 2])

                # ---- AllGather layer-2 table; widen rows to 128 ----
                nc.gpsimd.collective_compute(
                    "AllGather", ALU.bypass,
                    replica_groups=[list(range(cfg.NC))],
                    ins=[h2own_c[:]], outs=[table2s[:]])
                nc.sync.dma_start(out=table2[:, 0:48], in_=table2s[:])
                tc.strict_bb_all_engine_barrier()

                # ============ stage D/E: layer-2 edges =======================
                NCL = cfg.NCLASS
                for g, (glo, ghi) in enumerate(groups):
                    C = int(Call[g])
                    ncol = int(NCOL[g])
                    cs = [int(v) for v in CS[g]]
                    c01 = cs[0] + cs[1]
                    idx, ll = load_idx(g)
                    o = 0
                    o_s1lo = o; o += c01 // 16
                    o_s1hi = o; o += (cs[2] + cs[3]) // 16
                    o_dst = o; o += C // 16
                    o_s2 = []
                    for sec in range(4):
                        o_s2.append(o); o += cs[sec] // 16
                    g2 = gxp.tile([128, NCOLMAX, cfg.T2W], DT_BF16, tag="gd")
                    cb = 0
                    for sec in range(4):
                        if cs[sec] == 0:
                            continue
                        tv = table2[0:min(HALF, cfg.NROW2), :] if sec in (0, 2) \
                            else table2[HALF:cfg.NROW2, :]
                        _gather_chunked(nc, g2, cb, tv, idx, o_s2[sec],
                                        cs[sec], cfg.T2W)
                        cb += cs[sec]
                    aesb = build_aedge(g, glo, ghi, adst2_sb, 1, False)
                    lg = smp.tile([128, NCOLMAX, 1], DT_BF16, tag="lg")
                    nc.vector.tensor_tensor(
                        out=lg[:, :ncol, :],
                        in0=g2[:, :ncol, NCL:NCL + 1],
                        in1=aesb[:, :ncol, 0:1], op=ALU.add)
                    lr = smp.tile([128, NCOLMAX, 1], DT_BF16, tag="lr")
                    nc.vector.tensor_scalar_mul(
                        out=lr[:, :ncol, :], in0=lg[:, :ncol, :], scalar1=0.2)
                    nc.vector.tensor_tensor(
                        out=lr[:, :ncol, :], in0=lr[:, :ncol, :],
                        in1=lg[:, :ncol, :], op=ALU.max)
                    rhs = rhp.tile([128, NCOLMAX, NCL + 1], DT_BF16, tag="rhs")
                    nc.scalar.activation(
                        out=rhs[:, :ncol, NCL:], in_=lr[:, :ncol, :],
                        func=AF.Exp)
                    nc.vector.tensor_tensor(
                        out=rhs[:, :ncol, 0:NCL],
                        in0=g2[:, :ncol, 0:NCL],
                        in1=rhs[:, :ncol, NCL:NCL + 1]
                            .to_broadcast([128, ncol, NCL]),
                        op=ALU.mult)
                    for t in range(glo, ghi):
                        M, cols, nct = build_M(g, t, glo, ll, False)
                        pseg = ps1.tile([128, NCL + 1], DT_F32, tag="pseg")
                        for i, (mc, c) in enumerate(cols):
                            nc.tensor.matmul(
                                out=pseg[:], lhsT=M[:, mc, :], rhs=rhs[:, c, :],
                                start=(i == 0), stop=(i == len(cols) - 1))
                        rec = ccp.tile([128, 1], DT_F32, tag="rec")
                        nc.vector.reciprocal(out=rec[:], in_=pseg[:, NCL:])
                        nc.vector.tensor_tensor(
                            out=outf_sb[:, t, :],
                            in0=pseg[:, 0:NCL],
                            in1=rec[:].to_broadcast([128, NCL]),
                            op=ALU.mult)
                        nc.vector.tensor_tensor(
                            out=outf_sb[:, t, :], in0=outf_sb[:, t, :],
                            in1=b2_sb[:], op=ALU.add)

            # ============ stage F: log_softmax ===========================
            with tc.tile_pool(name="fin", bufs=1) as fpp:
                mx = fpp.tile([128, NDT, 1], DT_F32, tag="mx")
                nc.vector.tensor_reduce(
                    out=mx[:], in_=outf_sb[:], axis=mybir.AxisListType.X,
                    op=ALU.max)
                ex = fpp.tile([128, NDT, cfg.NCLASS], DT_F32, tag="ex")
                nc.vector.tensor_tensor(
                    out=ex[:], in0=outf_sb[:],
                    in1=mx[:].to_broadcast([128, NDT, cfg.NCLASS]),
                    op=ALU.subtract)
                nc.scalar.activation(out=ex[:], in_=ex[:], func=AF.Exp)
                sm = fpp.tile([128, NDT, 1], DT_F32, tag="sm")
                nc.vector.tensor_reduce(
                    out=sm[:], in_=ex[:], axis=mybir.AxisListType.X,
                    op=ALU.add)
                nc.scalar.activation(out=sm[:], in_=sm[:], func=AF.Ln)
                nc.vector.tensor_tensor(
                    out=sm[:], in0=sm[:], in1=mx[:], op=ALU.add)
                nc.vector.tensor_tensor(
                    out=outf_sb[:], in0=outf_sb[:],
                    in1=sm[:].to_broadcast([128, NDT, cfg.NCLASS]),
                    op=ALU.subtract)
                nc.sync.dma_start(
                    out=out_d[:].rearrange("(t p) c -> p t c", p=128),
                    in_=outf_sb[:])
                if dbg:
                    nc.sync.dma_start(out=dbg_t1[:], in_=table1[:])
                    nc.sync.dma_start(out=dbg_t2[:], in_=table2[:])
    nc.compile()
    return nc


def _run(cfg, inputs, trace=False):
    meta, blob, llocb, lloctb = _prep_edges(cfg, np.asarray(inputs["edge_index"]))
    W1cat, W2cat, b1rep, b2rep = _prep_weights(
        cfg, inputs["W1"], inputs["att_src1"], inputs["att_dst1"], inputs["b1"],
        inputs["W2"], inputs["att_src2"], inputs["att_dst2"], inputs["b2"])
    x = np.asarray(inputs["x"], np.float32)
    xTf = np.zeros((cfg.NFEAT, cfg.NA), BF16)
    xTf[:, :cfg.N] = x.T.astype(BF16)
    iota = np.broadcast_to(np.arange(cfg.GG * 128, dtype=np.float32),
                           (128, cfg.GG * 128)).astype(BF16).copy()
    iota2 = (np.arange(128, dtype=np.float32)[:, None]
             + 128.0 * np.arange(cfg.GG, dtype=np.float32)[None, :]
             ).astype(BF16).copy()
    ident = np.eye(128, dtype=np.float32)

    nc = build_bass(cfg, meta)

    in_maps = []
    for k in range(cfg.NC):
        own = np.arange(k * cfg.SHARD, (k + 1) * cfg.SHARD)
        other = np.concatenate([np.arange(0, k * cfg.SHARD),
                                np.arange((k + 1) * cfg.SHARD, cfg.N)])
        perm = np.concatenate([own, other])
        xk = np.zeros((cfg.NFEAT, cfg.NA), BF16)
        xk[:, :cfg.N] = xTf[:, perm]
        in_maps.append(dict(
            xT=xk, w1cat=W1cat, w2cat=W2cat, b1rep=b1rep, b2rep=b2rep,
            iota=iota, iota2=iota2, ident=ident, blob=blob[k],
            lloc=llocb[k], lloct=lloctb[k]))

    res = run_bass_kernel_spmd(nc, in_maps, list(range(cfg.NC)), trace=trace)
    outs = [res.results[k]["out"][:cfg.SHARD] for k in range(cfg.NC)]
    full = np.concatenate(outs, axis=0)[:cfg.N].astype(np.float32)
    return full, res


def kernel(**inputs):
    cfg = Cfg()
    out, _ = _run(cfg, inputs, trace=False)
    return out



# revision 24
# speedup vs baseline: 1.8091x; 1.0312x over previous
"""GAT (2-layer, PyG-style) Bass kernel for Trainium2, 8 NeuronCores.

Sharding: 1D destination-node partition. Each core owns N/8 dst nodes; edges
are bucketed by dst so segment-softmax and scatter-add are local. Layer-1
node features (h | a_src | a_dst) are computed redundantly per core into a
per-core-permuted table (own shard first, so dst-side rows fit int16); the
layer-2 table is shard-computed and AllGathered.

Per-edge row fetches use dma_gather (int16 indices, 256B-multiple rows);
src-side indices >= 32768 are handled by a per-group section split over two
table views. Segmented softmax + scatter-add go through a one-hot indicator
matmul on the tensor engine.
"""

import os
import sys

sys.path.insert(0, "/opt/trn_rl_repo")

import numpy as np
import ml_dtypes

BF16 = ml_dtypes.bfloat16

from concourse import bacc, bass, mybir, tile
from concourse.bass_utils import run_bass_kernel_spmd

AF = mybir.ActivationFunctionType
ALU = mybir.AluOpType
DT_BF16 = mybir.dt.bfloat16
DT_F32 = mybir.dt.float32
DT_I16 = mybir.dt.int16
HALF = 32768


class Cfg:
    def __init__(self, N=50000, E=800000):
        self.N = N
        self.E = E
        self.NC = 8
        self.NFEAT = 256
        self.NHID = 16
        self.HEADS = 8
        self.NCLASS = 40
        self.FH = self.HEADS * self.NHID          # 128
        self.T1W = 256                            # table1 row: h|a_src|a_dst|pad
        self.T2W = 128                            # table2 row: h2|a_src2|a_dst2|pad
        assert N % self.NC == 0
        self.SHARD = N // self.NC
        self.NDT = (self.SHARD + 127) // 128      # dst tiles per core
        self.LSH = self.NDT * 128
        self.NROW2 = self.NC * self.LSH
        self.ABATCH = 3
        self.NA = ((N + 384 - 1) // 384) * 384    # stage-A padded rows
        self.GG = 2                               # dst tiles per gather group



CHUNK = 1024


def _gather_chunked(nc, out_tile, col0, table_view, idx_tile, o16, total, elem):
    """Emit dma_gather calls of <=CHUNK idxs; out columns start at col0."""
    done = 0
    while done < total:
        n = min(CHUNK, total - done)
        nc.gpsimd.dma_gather(
            out_tile[:, (col0 + done) // 128:(col0 + done + n) // 128, :],
            table_view,
            idx_tile[:, o16 + done // 16:o16 + (done + n) // 16],
            n, n, elem)
        done += n

def _wrap16(vals):
    """int16 values -> dma_gather wrapped layout [128, len/16]."""
    n = len(vals)
    assert n % 16 == 0
    w = np.asarray(vals, np.int16).reshape(n // 16, 16).T  # [16, n/16]
    return np.tile(w, (8, 1))                              # [128, n/16]


def _prep_edges(cfg, edge_index):
    """Per-core, per-group edge layout with 4 sections by
    (src row>=32768 in table1, src row>=32768 in table2)."""
    N, NC, NDT, SHARD, LSH, GG = (cfg.N, cfg.NC, cfg.NDT, cfg.SHARD,
                                  cfg.LSH, cfg.GG)
    src = np.concatenate([np.asarray(edge_index[0]), np.arange(N)]).astype(np.int64)
    dst = np.concatenate([np.asarray(edge_index[1]), np.arange(N)]).astype(np.int64)
    core = dst // SHARD
    ldst = dst - core * SHARD                      # local dst 0..SHARD-1
    row2 = src + (src // SHARD) * (LSH - SHARD)    # table2 row of src
    groups = [(g, min(g + GG, NDT)) for g in range(0, NDT, GG)]
    NG = len(groups)
    tl = ldst // 128
    gl = np.searchsorted(np.array([a for a, _ in groups]), tl, side="right") - 1

    # per-core src row in the permuted table1: own shard first, others by id
    row1 = np.empty((NC, N), np.int64)
    for k in range(NC):
        own = np.arange(k * SHARD, (k + 1) * SHARD)
        other = np.concatenate([np.arange(0, k * SHARD),
                                np.arange((k + 1) * SHARD, N)])
        perm = np.concatenate([own, other])
        inv = np.empty(N, np.int64)
        inv[perm] = np.arange(N)
        row1[k] = inv

    key = core * NG + gl
    order = np.argsort(key, kind="stable")
    ks = key[order]
    bounds = np.searchsorted(ks, np.arange(NC * NG + 1))

    # section membership per (core, edge): 2*(row1>=H) + (row2>=H)
    # first pass: section counts -> shared section sizes CS[g, s]
    secs = [[None] * NG for _ in range(NC)]
    cnt = np.zeros((NC, NG, 4), np.int64)
    for k in range(NC):
        for g in range(NG):
            ids = order[bounds[k * NG + g]:bounds[k * NG + g + 1]]
            s = 2 * (row1[k][src[ids]] >= HALF) + (row2[ids] >= HALF)
            # order: section, then tile, stable
            o2 = np.lexsort((tl[ids], s))
            ids = ids[o2]
            s = s[o2]
            secs[k][g] = (ids, s)
            cnt[k, g] = np.bincount(s, minlength=4)
    CS = (-(-cnt.max(axis=0) // 128) * 128).astype(np.int64)  # [NG, 4]
    Call = CS.sum(axis=1)                                     # slots per group
    NCOL = (Call // 128).astype(np.int64)

    # blob layout per group (int16 cols): src1lo, src1hi, dst, s2_0..3
    # col counts: (CS0+CS1)/16, (CS2+CS3)/16, Call/16, CS0/16.. CS3/16
    blob_cols = ((CS[:, 0] + CS[:, 1]) + (CS[:, 2] + CS[:, 3])
                 + Call + Call) // 16
    blob_off = np.concatenate([[0], np.cumsum(blob_cols)]).astype(int)
    BLOBTOT = int(blob_off[-1])
    lloc_off = np.concatenate([[0], np.cumsum(NCOL)]).astype(int)
    LLTOT = int(lloc_off[-1])

    blob = np.zeros((NC, 128, BLOBTOT), np.int16)
    llocb = np.full((NC, 128, LLTOT), 1000.0, np.float32)
    lloctb = np.full((NC, LLTOT, 128), 1000.0, np.float32)

    # per (group, tile): column ranges [(c0, c1), ...] in group-local columns
    tile_ranges = [[[] for _ in range(NDT)] for _ in range(NG)]

    for k in range(NC):
        for g, (glo, ghi) in enumerate(groups):
            ids, s = secs[k][g]
            # slot position: section base + within-section position
            sbase = np.concatenate([[0], np.cumsum(CS[g])])[:4]
            pos = np.empty(len(ids), np.int64)
            for sec in range(4):
                m = s == sec
                pos[m] = sbase[sec] + np.arange(m.sum())
            C = int(Call[g])
            # index arrays, padded with 0 (valid row 0)
            r1 = np.zeros(C, np.int64)
            r2v = np.zeros(C, np.int64)
            dl = np.zeros(C, np.int64)
            lv = np.full(C, 1000.0, np.float32)
            r1[pos] = row1[k][src[ids]]
            r2v[pos] = row2[ids]
            dl[pos] = ldst[ids]
            lv[pos] = (ldst[ids] - glo * 128).astype(np.float32)
            c01 = int(CS[g, 0] + CS[g, 1])
            seg = []
            seg.append(_wrap16(r1[:c01]))                        # src1 low
            seg.append(_wrap16(r1[c01:] - HALF * (r1[c01:] >= HALF)))  # src1 hi
            seg.append(_wrap16(dl))                              # dst (both layers)
            cb = np.concatenate([[0], np.cumsum(CS[g])]).astype(int)
            for sec in range(4):
                v = r2v[cb[sec]:cb[sec + 1]]
                seg.append(_wrap16(v - HALF * (v >= HALF)))
            blob[k, :, blob_off[g]:blob_off[g + 1]] = np.concatenate(seg, axis=1)
            llocb[k, :, lloc_off[g]:lloc_off[g + 1]] = \
                lv.reshape(int(NCOL[g]), 128).T
            lloctb[k, lloc_off[g]:lloc_off[g + 1], :] = \
                lv.reshape(int(NCOL[g]), 128)
            if k == 0:
                # column ranges per tile (shared: derive from all cores below)
                pass
    # tile column ranges: union over cores of occupied columns per (g, t)
    occ = np.zeros((NG, NDT, 1), object)
    for g, (glo, ghi) in enumerate(groups):
        ncol = int(NCOL[g])
        used = np.zeros((NDT, ncol), bool)
        for k in range(NC):
            ids, s = secs[k][g]
            sbase = np.concatenate([[0], np.cumsum(CS[g])])[:4]
            pos = np.empty(len(ids), np.int64)
            for sec in range(4):
                m = s == sec
                pos[m] = sbase[sec] + np.arange(m.sum())
            t_of = tl[ids]
            for t in range(glo, ghi):
                cols = np.unique(pos[t_of == t] // 128)
                used[t, cols] = True
        for t in range(glo, ghi):
            cols = np.where(used[t])[0]
            ranges = []
            if len(cols):
                brk = np.where(np.diff(cols) > 1)[0]
                st = 0
                for b in list(brk) + [len(cols) - 1]:
                    ranges.append((int(cols[st]), int(cols[b]) + 1))
                    st = b + 1
            tile_ranges[g][t] = ranges

    meta = dict(groups=groups, CS=CS, Call=Call, NCOL=NCOL,
                blob_off=blob_off, BLOBTOT=BLOBTOT,
                lloc_off=lloc_off, LLTOT=LLTOT, tile_ranges=tile_ranges)
    return meta, blob, llocb.astype(BF16), lloctb.astype(BF16)


def _prep_weights(cfg, W1, att_src1, att_dst1, b1, W2, att_src2, att_dst2, b2):
    W1 = np.asarray(W1, np.float32)
    A1 = np.zeros((cfg.FH, 2 * cfg.HEADS), np.float32)
    for h in range(cfg.HEADS):
        A1[h * cfg.NHID:(h + 1) * cfg.NHID, h] = np.asarray(att_src1)[h]
        A1[h * cfg.NHID:(h + 1) * cfg.NHID, cfg.HEADS + h] = np.asarray(att_dst1)[h]
    W1cat = np.concatenate([W1, W1 @ A1], axis=1).astype(BF16)  # [NFEAT, 144]
    W2cat = np.zeros((cfg.FH, 48), np.float32)
    W2cat[:, :cfg.NCLASS] = np.asarray(W2)
    W2cat[:, cfg.NCLASS] = np.asarray(W2) @ np.asarray(att_src2)[0]
    W2cat[:, cfg.NCLASS + 1] = np.asarray(W2) @ np.asarray(att_dst2)[0]
    W2cat = W2cat.astype(BF16)
    b1rep = np.broadcast_to(np.asarray(b1, np.float32), (128, cfg.FH)).copy()
    b2rep = np.broadcast_to(np.asarray(b2, np.float32), (128, cfg.NCLASS)).copy()
    return W1cat, W2cat, b1rep, b2rep


def build_bass(cfg, meta):
    nc = bacc.Bacc("TRN2", target_bir_lowering=False, debug=False)
    NDT, GG = cfg.NDT, cfg.GG
    TBW = cfg.FH + 2 * cfg.HEADS                   # 144 live cols of table1
    NAB = cfg.NA // (128 * cfg.ABATCH)
    groups = meta["groups"]
    CS, Call, NCOL = meta["CS"], meta["Call"], meta["NCOL"]
    blob_off, lloc_off = meta["blob_off"], meta["lloc_off"]
    tile_ranges = meta["tile_ranges"]
    NCOLMAX = int(max(NCOL))
    BLOBMAX = int(max(blob_off[i + 1] - blob_off[i] for i in range(len(groups))))
    CMAX_T = 1
    for g in range(len(groups)):
        for t in range(NDT):
            if tile_ranges[g][t]:
                CMAX_T = max(CMAX_T,
                             sum(b - a for (a, b) in tile_ranges[g][t]))

    xT = nc.dram_tensor("xT", [cfg.NFEAT, cfg.NA], DT_BF16, kind="ExternalInput")
    w1cat = nc.dram_tensor("w1cat", [cfg.NFEAT, TBW], DT_BF16, kind="ExternalInput")
    w2cat = nc.dram_tensor("w2cat", [cfg.FH, 48], DT_BF16, kind="ExternalInput")
    b1rep_d = nc.dram_tensor("b1rep", [128, cfg.FH], DT_F32, kind="ExternalInput")
    b2rep_d = nc.dram_tensor("b2rep", [128, cfg.NCLASS], DT_F32, kind="ExternalInput")
    iota_d = nc.dram_tensor("iota", [128, GG * 128], DT_BF16, kind="ExternalInput")
    iota2_d = nc.dram_tensor("iota2", [128, GG], DT_BF16, kind="ExternalInput")
    ident_d = nc.dram_tensor("ident", [128, 128], DT_F32, kind="ExternalInput")
    blob_d = nc.dram_tensor("blob", [128, meta["BLOBTOT"]], DT_I16,
                            kind="ExternalInput")
    lloc_d = nc.dram_tensor("lloc", [128, meta["LLTOT"]], DT_BF16,
                            kind="ExternalInput")
    lloct_d = nc.dram_tensor("lloct", [meta["LLTOT"], 128], DT_BF16,
                             kind="ExternalInput")
    out_d = nc.dram_tensor("out", [cfg.LSH, cfg.NCLASS], DT_F32,
                           kind="ExternalOutput")

    table1 = nc.dram_tensor("table1", [cfg.NA, cfg.T1W], DT_BF16)
    h2own_c = nc.dram_tensor("h2own_c", [cfg.LSH, 48], DT_BF16)
    # stage-B-built one-hot matrices, reloaded in stage D instead of rebuilt
    moff = {}
    off = 0
    for g in range(len(groups)):
        for t in range(NDT):
            w = sum(b - a for (a, b) in tile_ranges[g][t])
            if w:
                moff[(g, t)] = off
                off += w
    MTOT = max(off, 1)
    m_dram = nc.dram_tensor("m_dram", [128, MTOT, 128], DT_BF16)
    mt_dram = nc.dram_tensor("mt_dram", [128, MTOT, 128], DT_BF16)
    table2s = nc.dram_tensor("table2s", [cfg.NROW2, 48], DT_BF16,
                             addr_space="Shared")
    table2 = nc.dram_tensor("table2", [cfg.NROW2, cfg.T2W], DT_BF16)

    dbg = os.environ.get("GAT_DEBUG_DUMP") == "1"
    if dbg:
        dbg_t1 = nc.dram_tensor("dbg_t1", [cfg.NA, cfg.T1W], DT_BF16,
                                kind="ExternalOutput")
        dbg_t2 = nc.dram_tensor("dbg_t2", [cfg.NROW2, cfg.T2W], DT_BF16,
                                kind="ExternalOutput")
        dbg_ps = nc.dram_tensor("dbg_ps", [cfg.LSH, TBW], DT_F32,
                                kind="ExternalOutput")
        dbg_g1 = nc.dram_tensor("dbg_g1", [128, NCOLMAX, cfg.T1W], DT_BF16,
                                kind="ExternalOutput")

    with tile.TileContext(nc) as tc:
        with tc.tile_pool(name="const", bufs=1) as cpool:
            w1_sb = cpool.tile([128, cfg.NFEAT // 128, TBW], DT_BF16)
            nc.sync.dma_start(out=w1_sb[:],
                              in_=w1cat[:].rearrange("(kt p) c -> p kt c", p=128))
            w2_sb = cpool.tile([128, 48], DT_BF16)
            nc.sync.dma_start(out=w2_sb[:], in_=w2cat[:])
            b1_sb = cpool.tile([128, cfg.FH], DT_F32)
            nc.sync.dma_start(out=b1_sb[:], in_=b1rep_d[:])
            b2_sb = cpool.tile([128, cfg.NCLASS], DT_F32)
            nc.sync.dma_start(out=b2_sb[:], in_=b2rep_d[:])
            iota_sb = cpool.tile([128, GG * 128], DT_BF16)
            nc.sync.dma_start(out=iota_sb[:], in_=iota_d[:])
            iota2_sb = cpool.tile([128, GG], DT_BF16)
            nc.sync.dma_start(out=iota2_sb[:], in_=iota2_d[:])
            ident_sb = cpool.tile([128, 128], DT_F32)
            nc.sync.dma_start(out=ident_sb[:], in_=ident_d[:])
            outf_sb = cpool.tile([128, NDT, cfg.NCLASS], DT_F32)
            adst1_sb = cpool.tile([128, NDT, cfg.HEADS], DT_BF16)
            adst2_sb = cpool.tile([128, NDT, 1], DT_BF16)

            # ============ stage A: table1 + local a_dst table ================
            with (
                tc.tile_pool(name="ax", bufs=3) as axp,
                tc.tile_pool(name="atb", bufs=3) as atbp,
                tc.tile_pool(name="apsum", bufs=2, space="PSUM") as app,
            ):
                for bidx in range(NAB):
                    n0 = bidx * 128 * cfg.ABATCH
                    xt = axp.tile([128, cfg.NFEAT // 128, 128 * cfg.ABATCH],
                                  DT_BF16, tag="xt")
                    nc.scalar.dma_start(
                        out=xt[:],
                        in_=xT[:, n0:n0 + 128 * cfg.ABATCH].rearrange(
                            "(kt p) c -> p kt c", p=128))
                    pa = app.tile([128, cfg.ABATCH * TBW], DT_F32, tag="pa")
                    for m in range(cfg.ABATCH):
                        for kt in range(cfg.NFEAT // 128):
                            nc.tensor.matmul(
                                out=pa[:, m * TBW:(m + 1) * TBW],
                                lhsT=xt[:, kt, m * 128:(m + 1) * 128],
                                rhs=w1_sb[:, kt, :],
                                start=(kt == 0),
                                stop=(kt == cfg.NFEAT // 128 - 1))
                    tb = atbp.tile([128, cfg.ABATCH * TBW], DT_BF16, tag="tb")
                    nc.scalar.copy(out=tb[:], in_=pa[:])
                    nc.sync.dma_start(
                        out=table1[n0:n0 + 128 * cfg.ABATCH, 0:TBW].rearrange(
                            "(m p) c -> p m c", p=128),
                        in_=tb[:].rearrange("p (m c) -> p m c", c=TBW))
                    # local a_dst rows (a_dst = psum cols FH+8 : FH+16)
                    for m in range(cfg.ABATCH):
                        r0 = n0 + m * 128
                        if r0 >= cfg.LSH:
                            break
                        nc.vector.tensor_copy(
                            out=adst1_sb[:, r0 // 128, :],
                            in_=pa[:, m * TBW + cfg.FH + cfg.HEADS:
                                   m * TBW + cfg.FH + 2 * cfg.HEADS])

            tc.strict_bb_all_engine_barrier()

            # ============ stage B/C: layer-1 edges + layer-2 table ===========
            with (
                tc.tile_pool(name="gx", bufs=2) as gxp,
                tc.tile_pool(name="gi", bufs=2) as gip,
                tc.tile_pool(name="mm", bufs=2) as mmp,
                tc.tile_pool(name="lt", bufs=2) as ltp,
                tc.tile_pool(name="mt", bufs=2) as mtp,
                tc.tile_pool(name="rh", bufs=2) as rhp,
                tc.tile_pool(name="sm", bufs=3) as smp,
                tc.tile_pool(name="cc", bufs=2) as ccp,
                tc.tile_pool(name="ps1", bufs=2, space="PSUM") as ps1,
                tc.tile_pool(name="ps2", bufs=2, space="PSUM") as ps2,
                tc.tile_pool(name="ps3", bufs=2, space="PSUM") as ps3,
                tc.tile_pool(name="pae", bufs=2, space="PSUM") as pae,
            ):
                def load_idx(g):
                    bo = int(blob_off[g])
                    bw = int(blob_off[g + 1]) - bo
                    idx = gip.tile([128, BLOBMAX], DT_I16, tag="idx")
                    nc.sync.dma_start(out=idx[:, :bw], in_=blob_d[:, bo:bo + bw])
                    ll = gip.tile([128, NCOLMAX], DT_BF16, tag="ll")
                    lo = int(lloc_off[g])
                    lw = int(lloc_off[g + 1]) - lo
                    nc.sync.dma_start(out=ll[:, :lw], in_=lloc_d[:, lo:lo + lw])
                    return idx, ll

                def col_tiles_of(g, glo, ghi):
                    """column -> [(tile, local col in that tile's MT)]"""
                    seq = {}
                    for t in range(glo, ghi):
                        cpos = 0
                        for (a, b) in tile_ranges[g][t]:
                            for c in range(a, b):
                                seq.setdefault(c, []).append((t, cpos + c - a))
                            cpos += b - a
                    return seq

                def build_aedge(g, glo, ghi, adst_sb, width, save):
                    """per-edge a_dst via transposed one-hot matmul:
                    ae[p, c, :] = adst[dst(p, c), :]"""
                    lo = int(lloc_off[g])
                    ncol = int(NCOL[g])
                    if save:
                        ltr = ltp.tile([128, NCOLMAX, 128], DT_BF16, tag="ltr")
                        nc.sync.dma_start(
                            out=ltr[:, :ncol, :],
                            in_=lloct_d[lo:lo + ncol, :].unsqueeze(0)
                                .to_broadcast([128, ncol, 128]))
                    MTs = {}
                    for t in range(glo, ghi):
                        ranges = tile_ranges[g][t]
                        if not ranges:
                            continue
                        nct = sum(b - a for (a, b) in ranges)
                        MT = mtp.tile([128, max(CMAX_T, 1), 128], DT_BF16,
                                      tag=f"MT{t - glo}")
                        if save:
                            cpos = 0
                            for (a, b) in ranges:
                                w = b - a
                                nc.vector.tensor_tensor(
                                    out=MT[:, cpos:cpos + w, :],
                                    in0=ltr[:, a:b, :],
                                    in1=iota2_sb[:, t - glo:t - glo + 1]
                                        .to_broadcast([128, w, 128]),
                                    op=ALU.is_equal)
                                cpos += w
                            nc.sync.dma_start(
                                out=mt_dram[:, moff[(g, t)]:moff[(g, t)] + nct,
                                            :],
                                in_=MT[:, :nct, :])
                        else:
                            nc.sync.dma_start(
                                out=MT[:, :nct, :],
                                in_=mt_dram[:, moff[(g, t)]:moff[(g, t)] + nct,
                                            :])
                        MTs[t] = MT
                    ae = pae.tile([128, NCOLMAX, 8], DT_F32, tag="ae")
                    for c, lst in sorted(col_tiles_of(g, glo, ghi).items()):
                        for i, (t, mc) in enumerate(lst):
                            nc.tensor.matmul(
                                out=ae[:, c, 0:width],
                                lhsT=MTs[t][:, mc, :],
                                rhs=adst_sb[:, t, 0:width],
                                start=(i == 0), stop=(i == len(lst) - 1))
                    aesb = smp.tile([128, NCOLMAX, 8], DT_BF16, tag="aesb")
                    nc.scalar.copy(out=aesb[:, :ncol, 0:width],
                                   in_=ae[:, :ncol, 0:width])
                    return aesb

                def build_M(g, t, glo, ll, save):
                    ranges = tile_ranges[g][t]
                    ncols_t = sum(b - a for (a, b) in ranges)
                    M = mmp.tile([128, max(CMAX_T, 1), 128], DT_BF16, tag="M")
                    cpos = 0
                    cols = []
                    for (a, b) in ranges:
                        w = b - a
                        if save:
                            nc.vector.tensor_tensor(
                                out=M[:, cpos:cpos + w, :],
                                in0=ll[:, a:b].to_broadcast([128, w, 128]),
                                in1=iota_sb[:,
                                            (t - glo) * 128:(t - glo + 1) * 128]
                                    .unsqueeze(1).to_broadcast([128, w, 128]),
                                op=ALU.is_equal)
                        for c in range(a, b):
                            cols.append((cpos + c - a, c))
                        cpos += w
                    if ncols_t:
                        if save:
                            nc.sync.dma_start(
                                out=m_dram[:, moff[(g, t)]:moff[(g, t)]
                                           + ncols_t, :],
                                in_=M[:, :ncols_t, :])
                        else:
                            nc.sync.dma_start(
                                out=M[:, :ncols_t, :],
                                in_=m_dram[:, moff[(g, t)]:moff[(g, t)]
                                           + ncols_t, :])
                    return M, cols, ncols_t

                for g, (glo, ghi) in enumerate(groups):
                    C = int(Call[g])
                    ncol = int(NCOL[g])
                    cs = [int(v) for v in CS[g]]
                    c01 = cs[0] + cs[1]
                    c23 = cs[2] + cs[3]
                    idx, ll = load_idx(g)
                    # offsets into idx blob (cols of 16 idx each)
                    o = 0
                    o_s1lo = o; o += c01 // 16
                    o_s1hi = o; o += c23 // 16
                    o_dst = o; o += C // 16
                    o_s2 = []
                    for sec in range(4):
                        o_s2.append(o); o += cs[sec] // 16
                    g1 = gxp.tile([128, NCOLMAX, cfg.T1W], DT_BF16, tag="g1")
                    if c01:
                        _gather_chunked(nc, g1, 0,
                                        table1[0:min(HALF, cfg.NA), :],
                                        idx, o_s1lo, c01, cfg.T1W)
                    if c23:
                        _gather_chunked(nc, g1, c01, table1[HALF:cfg.NA, :],
                                        idx, o_s1hi, c23, cfg.T1W)
                    aesb = build_aedge(g, glo, ghi, adst1_sb, cfg.HEADS, True)
                    if dbg and g == 0:
                        nc.sync.dma_start(out=dbg_g1[:, :ncol, :],
                                          in_=g1[:, :ncol, :])
                    # edgewise: logits -> leaky -> exp ; messages
                    lg = smp.tile([128, NCOLMAX, cfg.HEADS], DT_BF16, tag="lg")
                    nc.vector.tensor_tensor(
                        out=lg[:, :ncol, :],
                        in0=g1[:, :ncol, cfg.FH:cfg.FH + cfg.HEADS],
                        in1=aesb[:, :ncol, 0:cfg.HEADS], op=ALU.add)
                    lr = smp.tile([128, NCOLMAX, cfg.HEADS], DT_BF16, tag="lr")
                    nc.vector.tensor_scalar_mul(
                        out=lr[:, :ncol, :], in0=lg[:, :ncol, :], scalar1=0.2)
                    nc.vector.tensor_tensor(
                        out=lr[:, :ncol, :], in0=lr[:, :ncol, :],
                        in1=lg[:, :ncol, :], op=ALU.max)
                    rhs = rhp.tile([128, NCOLMAX, cfg.FH + cfg.HEADS], DT_BF16,
                                   tag="rhs")
                    nc.scalar.activation(
                        out=rhs[:, :ncol, cfg.FH:], in_=lr[:, :ncol, :],
                        func=AF.Exp)
                    for hh in range(cfg.HEADS):
                        nc.vector.tensor_tensor(
                            out=rhs[:, :ncol, hh * cfg.NHID:(hh + 1) * cfg.NHID],
                            in0=g1[:, :ncol, hh * cfg.NHID:(hh + 1) * cfg.NHID],
                            in1=rhs[:, :ncol, cfg.FH + hh:cfg.FH + hh + 1]
                                .to_broadcast([128, ncol, cfg.NHID]),
                            op=ALU.mult)
                    for t in range(glo, ghi):
                        M, cols, nct = build_M(g, t, glo, ll, True)
                        pseg = ps1.tile([128, TBW], DT_F32, tag="pseg")
                        for i, (mc, c) in enumerate(cols):
                            nc.tensor.matmul(
                                out=pseg[:, 0:cfg.FH + cfg.HEADS],
                                lhsT=M[:, mc, :], rhs=rhs[:, c, :],
                                start=(i == 0), stop=(i == len(cols) - 1))
                        if dbg:
                            psc = ccp.tile([128, TBW], DT_F32, tag="psc")
                            nc.vector.tensor_copy(out=psc[:], in_=pseg[:])
                            nc.sync.dma_start(
                                out=dbg_ps[t * 128:(t + 1) * 128, :], in_=psc[:])
                        # ---- stage C ----
                        rec = ccp.tile([128, cfg.HEADS], DT_F32, tag="rec")
                        nc.vector.reciprocal(
                            out=rec[:], in_=pseg[:, cfg.FH:cfg.FH + cfg.HEADS])
                        o1 = ccp.tile([128, cfg.FH], DT_F32, tag="o1")
                        nc.vector.tensor_tensor(
                            out=o1[:].rearrange("p (h c) -> p h c", c=cfg.NHID),
                            in0=pseg[:, 0:cfg.FH].rearrange(
                                "p (h c) -> p h c", c=cfg.NHID),
                            in1=rec[:].to_broadcast([128, cfg.HEADS, cfg.NHID]),
                            op=ALU.mult)
                        nc.vector.tensor_tensor(
                            out=o1[:], in0=o1[:], in1=b1_sb[:], op=ALU.add)
                        tmin = ccp.tile([128, cfg.FH], DT_F32, tag="tmin")
                        nc.vector.tensor_scalar_min(
                            out=tmin[:], in0=o1[:], scalar1=0.0)
                        nc.scalar.activation(out=tmin[:], in_=tmin[:],
                                             func=AF.Exp)
                        nc.vector.tensor_scalar_add(
                            out=tmin[:], in0=tmin[:], scalar1=-1.0)
                        a1t = ccp.tile([128, cfg.FH], DT_F32, tag="a1t")
                        nc.vector.tensor_tensor(
                            out=a1t[:], in0=o1[:], in1=tmin[:], op=ALU.max)
                        ptr = ps2.tile([128, 128], DT_F32, tag="ptr")
                        nc.tensor.transpose(out=ptr[:], in_=a1t[:],
                                            identity=ident_sb[:])
                        a1T = ccp.tile([128, 128], DT_BF16, tag="a1T")
                        nc.scalar.copy(out=a1T[:], in_=ptr[:])
                        ph2 = ps3.tile([128, 48], DT_F32, tag="ph2")
                        nc.tensor.matmul(out=ph2[:], lhsT=a1T[:], rhs=w2_sb[:],
                                         start=True, stop=True)
                        t2 = ccp.tile([128, 48], DT_BF16, tag="t2")
                        nc.scalar.copy(out=t2[:], in_=ph2[:])
                        nc.sync.dma_start(
                            out=h2own_c[t * 128:(t + 1) * 128, :], in_=t2[:])
                        nc.vector.tensor_copy(
                            out=adst2_sb[:, t, :],
                            in_=t2[:, cfg.NCLASS + 1:cfg.NCLASS + 2])

                # ---- AllGather layer-2 table; widen rows to 128 ----
                nc.gpsimd.collective_compute(
                    "AllGather", ALU.bypass,
                    replica_groups=[list(range(cfg.NC))],
                    ins=[h2own_c[:]], outs=[table2s[:]])
                nc.sync.dma_start(out=table2[:, 0:48], in_=table2s[:])
                tc.strict_bb_all_engine_barrier()

                # ============ stage D/E: layer-2 edges =======================
                NCL = cfg.NCLASS
                for g, (glo, ghi) in enumerate(groups):
                    C = int(Call[g])
                    ncol = int(NCOL[g])
                    cs = [int(v) for v in CS[g]]
                    c01 = cs[0] + cs[1]
                    idx, ll = load_idx(g)
                    o = 0
                    o_s1lo = o; o += c01 // 16
                    o_s1hi = o; o += (cs[2] + cs[3]) // 16
                    o_dst = o; o += C // 16
                    o_s2 = []
                    for sec in range(4):
                        o_s2.append(o); o += cs[sec] // 16
                    g2 = gxp.tile([128, NCOLMAX, cfg.T2W], DT_BF16, tag="gd")
                    cb = 0
                    for sec in range(4):
                        if cs[sec] == 0:
                            continue
                        tv = table2[0:min(HALF, cfg.NROW2), :] if sec in (0, 2) \
                            else table2[HALF:cfg.NROW2, :]
                        _gather_chunked(nc, g2, cb, tv, idx, o_s2[sec],
                                        cs[sec], cfg.T2W)
                        cb += cs[sec]
                    aesb = build_aedge(g, glo, ghi, adst2_sb, 1, False)
                    lg = smp.tile([128, NCOLMAX, 1], DT_BF16, tag="lg")
                    nc.vector.tensor_tensor(
                        out=lg[:, :ncol, :],
                        in0=g2[:, :ncol, NCL:NCL + 1],
                        in1=aesb[:, :ncol, 0:1], op=ALU.add)
                    lr = smp.tile([128, NCOLMAX, 1], DT_BF16, tag="lr")
                    nc.vector.tensor_scalar_mul(
                        out=lr[:, :ncol, :], in0=lg[:, :ncol, :], scalar1=0.2)
                    nc.vector.tensor_tensor(
                        out=lr[:, :ncol, :], in0=lr[:, :ncol, :],
                        in1=lg[:, :ncol, :], op=ALU.max)
                    rhs = rhp.tile([128, NCOLMAX, NCL + 1], DT_BF16, tag="rhs")
                    nc.scalar.activation(
                        out=rhs[:, :ncol, NCL:], in_=lr[:, :ncol, :],
                        func=AF.Exp)
                    nc.vector.tensor_tensor(
                        out=rhs[:, :ncol, 0:NCL],
                        in0=g2[:, :ncol, 0:NCL],
                        in1=rhs[:, :ncol, NCL:NCL + 1]
                            .to_broadcast([128, ncol, NCL]),
                        op=ALU.mult)
                    for t in range(glo, ghi):
                        M, cols, nct = build_M(g, t, glo, ll, False)
                        pseg = ps1.tile([128, NCL + 1], DT_F32, tag="pseg")
                        for i, (mc, c) in enumerate(cols):
                            nc.tensor.matmul(
                                out=pseg[:], lhsT=M[:, mc, :], rhs=rhs[:, c, :],
                                start=(i == 0), stop=(i == len(cols) - 1))
                        rec = ccp.tile([128, 1], DT_F32, tag="rec")
                        nc.vector.reciprocal(out=rec[:], in_=pseg[:, NCL:])
                        nc.vector.tensor_tensor(
                            out=outf_sb[:, t, :],
                            in0=pseg[:, 0:NCL],
                            in1=rec[:].to_broadcast([128, NCL]),
                            op=ALU.mult)
                        nc.vector.tensor_tensor(
                            out=outf_sb[:, t, :], in0=outf_sb[:, t, :],
                            in1=b2_sb[:], op=ALU.add)

            # ============ stage F: log_softmax ===========================
            with tc.tile_pool(name="fin", bufs=1) as fpp:
                mx = fpp.tile([128, NDT, 1], DT_F32, tag="mx")
                nc.vector.tensor_reduce(
                    out=mx[:], in_=outf_sb[:], axis=mybir.AxisListType.X,
                    op=ALU.max)
                ex = fpp.tile([128, NDT, cfg.NCLASS], DT_F32, tag="ex")
                nc.vector.tensor_tensor(
                    out=ex[:], in0=outf_sb[:],
                    in1=mx[:].to_broadcast([128, NDT, cfg.NCLASS]),
                    op=ALU.subtract)
                nc.scalar.activation(out=ex[:], in_=ex[:], func=AF.Exp)
                sm = fpp.tile([128, NDT, 1], DT_F32, tag="sm")
                nc.vector.tensor_reduce(
                    out=sm[:], in_=ex[:], axis=mybir.AxisListType.X,
                    op=ALU.add)
                nc.scalar.activation(out=sm[:], in_=sm[:], func=AF.Ln)
                nc.vector.tensor_tensor(
                    out=sm[:], in0=sm[:], in1=mx[:], op=ALU.add)
                nc.vector.tensor_tensor(
                    out=outf_sb[:], in0=outf_sb[:],
                    in1=sm[:].to_broadcast([128, NDT, cfg.NCLASS]),
                    op=ALU.subtract)
                nc.sync.dma_start(
                    out=out_d[:].rearrange("(t p) c -> p t c", p=128),
                    in_=outf_sb[:])
                if dbg:
                    nc.sync.dma_start(out=dbg_t1[:], in_=table1[:])
                    nc.sync.dma_start(out=dbg_t2[:], in_=table2[:])
    nc.compile()
    return nc


def _run(cfg, inputs, trace=False):
    meta, blob, llocb, lloctb = _prep_edges(cfg, np.asarray(inputs["edge_index"]))
    W1cat, W2cat, b1rep, b2rep = _prep_weights(
        cfg, inputs["W1"], inputs["att_src1"], inputs["att_dst1"], inputs["b1"],
        inputs["W2"], inputs["att_src2"], inputs["att_dst2"], inputs["b2"])
    x = np.asarray(inputs["x"], np.float32)
    xTf = np.zeros((cfg.NFEAT, cfg.NA), BF16)
    xTf[:, :cfg.N] = x.T.astype(BF16)
    iota = np.broadcast_to(np.arange(cfg.GG * 128, dtype=np.float32),
                           (128, cfg.GG * 128)).astype(BF16).copy()
    iota2 = (np.arange(128, dtype=np.float32)[:, None]
             + 128.0 * np.arange(cfg.GG, dtype=np.float32)[None, :]
             ).astype(BF16).copy()
    ident = np.eye(128, dtype=np.float32)

    nc = build_bass(cfg, meta)

    in_maps = []
    for k in range(cfg.NC):
        own = np.arange(k * cfg.SHARD, (k + 1) * cfg.SHARD)
        other = np.concatenate([np.arange(0, k * cfg.SHARD),
                                np.arange((k + 1) * cfg.SHARD, cfg.N)])
        perm = np.concatenate([own, other])
        xk = np.zeros((cfg.NFEAT, cfg.NA), BF16)
        xk[:, :cfg.N] = xTf[:, perm]
        in_maps.append(dict(
            xT=xk, w1cat=W1cat, w2cat=W2cat, b1rep=b1rep, b2rep=b2rep,
            iota=iota, iota2=iota2, ident=ident, blob=blob[k],
            lloc=llocb[k], lloct=lloctb[k]))

    res = run_bass_kernel_spmd(nc, in_maps, list(range(cfg.NC)), trace=trace)
    outs = [res.results[k]["out"][:cfg.SHARD] for k in range(cfg.NC)]
    full = np.concatenate(outs, axis=0)[:cfg.N].astype(np.float32)
    return full, res


def kernel(**inputs):
    cfg = Cfg()
    out, _ = _run(cfg, inputs, trace=False)
    return out

